# revision 13
# baseline (speedup 1.0000x reference)
"""Trainium2 Bass kernel for nn_AbsorberPathAggregator.

Contract: kernel(**inputs) takes the FULL unsharded inputs (as produced by
setup_inputs()) and returns the FULL [B, NE, OUT] float32 output.

Strategy (pure data parallel over B across 8 NeuronCores, 4 samples/core):
  - Host (numpy): pair enumeration (argsort of valid triu pairs), gathers,
    RBF features -> geom_inT [353, 1024] per core; ejkT [64, 1024]; pmask.
    This is cheap index/bookkeeping work; all FLOP-heavy MLPs run on device.
  - Device per core, all activations kept transposed [feature, row]:
      G = gm-MLP(geom_inT) masked by pmask              (geometry path)
      u = pe_W1[:64].T @ ejkT                           (pair part of layer 1)
      for ne in 0..63:  x1 = Silu(u + v[ne]) ; x2 = Silu(W2.T x1)
                        g3 = W3.T x2 ; agg[:, ne*4+b] = sum_p (g3+b3)*G
      out = (Silu(opW1.T agg + b1)).T opW2 + b2  -> DMA out
    Matmuls use float32r (full-rate fp32-reduced), Silu on ScalarE with the
    free per-partition bias operand, epilogue via the fused affine_mul_reduce
    DVE op.
"""

import sys

for _p in (
    "/root/.axon_site",
    "/root/.axon_site/_ro/trn_rl_repo",
    "/root/.axon_site/_ro/pypackages",
    "/opt/trn_rl_repo",
):
    if _p not in sys.path:
        sys.path.append(_p)

import numpy as np

from contextlib import ExitStack

import concourse.tile as tile
from concourse import bacc, mybir
from concourse.bass_utils import run_bass_kernel_spmd

F32 = mybir.dt.float32
F32R = mybir.dt.float32r
F16 = mybir.dt.float16
AF = mybir.ActivationFunctionType
ALU = mybir.AluOpType

B, N, H = 32, 64, 128
RBF_DIM, GH, SCATTER, OUT = 32, 256, 64, 256
CUTOFF, PMAX = 6.0, 256
ZEMB, EDIM, NE, PH = 32, 32, 64, 256
GIN = 2 * H + 3 * RBF_DIM + 1  # 353

N_CORES = 8
BPC = B // N_CORES  # 4 samples per core
R = BPC * PMAX  # 1024 rows per core

TRACE = False  # set by test harness for NTFF profiling
_CACHE = {}


# --------------------------------------------------------------------------
# host-side preprocessing (numpy, mirrors the reference's indexing exactly)
# --------------------------------------------------------------------------

def _rbf(x):
    centers = np.linspace(0.0, CUTOFF, RBF_DIM, dtype=np.float32)
    delta = CUTOFF / (RBF_DIM - 1)
    gamma = 1.0 / (delta * delta + 1e-12)
    d = x[..., None] - centers
    return np.exp((-gamma) * d * d).astype(np.float32)


def _norm(v):
    return np.sqrt((v * v).sum(-1) + np.float32(1e-12))


def _host_prep(h, z, pos, mask, z_emb, absorber_index):
    ai = int(absorber_index)
    pos0 = pos[:, ai][:, None, :]  # [B,1,3]
    r = _norm(pos - pos0)  # [B,N]
    valid = mask & (np.arange(N)[None, :] != ai) & (r <= np.float32(CUTOFF))
    ju, ku = np.triu_indices(N, k=1)
    pv = valid[:, ju] & valid[:, ku]  # [B, Np]
    order = np.argsort(~pv, axis=1, kind="stable")[:, :PMAX]
    pmask = np.take_along_axis(pv, order, axis=1)  # [B,PMAX]
    j_idx = np.where(pmask, ju[order], 0)
    k_idx = np.where(pmask, ku[order], 0)
    bidx = np.arange(B)[:, None]
    hj, hk = h[bidx, j_idx], h[bidx, k_idx]  # [B,P,H]
    posj, posk = pos[bidx, j_idx], pos[bidx, k_idx]  # [B,P,3]
    vj, vk, vjk = posj - pos0, posk - pos0, posk - posj
    r0j, r0k, rjk = _norm(vj), _norm(vk), _norm(vjk)
    uj = vj / np.maximum(r0j[..., None], np.float32(1e-8))
    uk = vk / np.maximum(r0k[..., None], np.float32(1e-8))
    cosang = np.clip((uj * uk).sum(-1, keepdims=True), -1.0, 1.0).astype(np.float32)
    geom_in = np.concatenate(
        [
            hj,
            hk,
            _rbf(np.minimum(r0j, np.float32(CUTOFF))),
            _rbf(np.minimum(r0k, np.float32(CUTOFF))),
            _rbf(np.minimum(rjk, np.float32(CUTOFF))),
            cosang,
        ],
        axis=-1,
    ).astype(np.float32)  # [B, P, 353]
    ejk = np.concatenate([z_emb[z[bidx, j_idx]], z_emb[z[bidx, k_idx]]], axis=-1)
    return geom_in, ejk.astype(np.float32), pmask


# --------------------------------------------------------------------------
# device kernel
# --------------------------------------------------------------------------

# bias-pack column indices (columns within the dpack bias block)
GM_B1, GM_B2, GM_B3, PE_B2, PE_B3, OP_B1, OP_B2 = 0, 2, 4, 5, 7, 8, 10
NBIAS = 12

# dpack [128, DCOLS]: rows 0:64 cols 0:R = ejkT; rows 64:128 cols 0:R = pmask
# broadcast; rows 0:64 cols R:R+256 = pe_W1[:64]; cols R+256:R+384 = vb1 (2 mh
# halves of 64); cols R+384 : R+384+NBIAS = bias pack.
D_PW1 = R
D_VB1 = R + 256
D_BIA = R + 384
DCOLS = R + 384 + NBIAS

# wpack [128, WCOLS] column layout: peW2(512) peW3(128) gmW1(768) gmW2(512)
# gmW3(128) opW1(256) opW2(512)
W_PEW2 = 0
W_PEW3 = 512
W_GMW1 = 640
W_GMW2 = 1408
W_GMW3 = 1920
W_OPW1 = 2048
W_OPW2 = 2304
WCOLS = 2816
W_SPLIT = 640  # first chunk (pe weights) loaded on sync, rest on gpsimd


def _build():
    nc = bacc.Bacc("TRN2", target_bir_lowering=False, debug=False, num_devices=N_CORES)

    ginT_d = nc.dram_tensor("ginT", [GIN, R], F32R, kind="ExternalInput").ap()
    pmask_d = nc.dram_tensor("pmask", [64, R], F32, kind="ExternalInput").ap()
    dpack_d = nc.dram_tensor("dpack", [128, DCOLS], F32R, kind="ExternalInput").ap()
    wpack_d = nc.dram_tensor("wpack", [128, WCOLS], F32R, kind="ExternalInput").ap()
    out_d = nc.dram_tensor("out", [2, 128, 256], F32, kind="ExternalOutput").ap()

    K1 = [(0, 128), (128, 128), (256, GIN - 256)]  # gm layer-1 k tiles

    with tile.TileContext(nc) as tc, ExitStack() as ctx:
        wp = ctx.enter_context(tc.tile_pool(name="wp", bufs=1))
        ap = ctx.enter_context(tc.tile_pool(name="ap", bufs=3))
        sp = ctx.enter_context(tc.tile_pool(name="sp", bufs=2))
        psB = ctx.enter_context(tc.tile_pool(name="psB", bufs=2, space="PSUM"))
        psS = ctx.enter_context(tc.tile_pool(name="psS", bufs=2, space="PSUM"))

        # ---- loads (spread across engine queues; dpack is critical) ----
        dpack = wp.tile([128, DCOLS], F32R, tag="dpack")
        nc.sync.dma_start(dpack[0:64, 0:R], dpack_d[0:64, 0:R])
        nc.sync.dma_start(dpack[:, R:DCOLS], dpack_d[:, R:DCOLS])
        wpack = wp.tile([128, WCOLS], F32R, tag="wpack")
        nc.sync.dma_start(wpack[:, 0:W_SPLIT], wpack_d[:, 0:W_SPLIT])
        nc.sync.dma_start(wpack[:, W_SPLIT:W_GMW2], wpack_d[:, W_SPLIT:W_GMW2])
        nc.sync.dma_start(wpack[:, W_GMW2:], wpack_d[:, W_GMW2:])
        pmask2t = wp.tile([64, R], F32, tag="pmask2t")
        nc.sync.dma_start(pmask2t[:], pmask_d)
        pmask2 = pmask2t[:]

        ejkT = dpack[0:64, 0:R]
        pw1e = [dpack[0:64, D_PW1 + m * 128 : D_PW1 + (m + 1) * 128] for m in range(2)]
        vb1 = [
            dpack[:, D_VB1 + m * NE : D_VB1 + (m + 1) * NE].bitcast(F32)
            for m in range(2)
        ]
        bia = dpack[:, D_BIA : D_BIA + NBIAS].bitcast(F32)

        def wtile(c0, ksz, m):
            return wpack[0:ksz, c0 + m * 128 : c0 + (m + 1) * 128]

        peW2 = [[wtile(W_PEW2 + k * 256, 128, m) for m in range(2)] for k in range(2)]
        peW3 = [wpack[0:128, W_PEW3 + k * 64 : W_PEW3 + (k + 1) * 64] for k in range(2)]
        gmW1 = [
            [wtile(W_GMW1 + k * 256, K1[k][1], m) for m in range(2)] for k in range(3)
        ]
        gmW2 = [[wtile(W_GMW2 + k * 256, 128, m) for m in range(2)] for k in range(2)]
        gmW3 = [wpack[0:128, W_GMW3 + k * 64 : W_GMW3 + (k + 1) * 64] for k in range(2)]
        opW1 = [wpack[0:64, W_OPW1 + m * 128 : W_OPW1 + (m + 1) * 128] for m in range(2)]
        opW2 = [[wtile(W_OPW2 + k * 256, 128, m) for m in range(2)] for k in range(2)]

        ginT = []
        for k, (k0, ksz) in enumerate(K1):
            t = wp.tile([ksz, R], F32R, tag=f"ginT{k}", name=f"ginT{k}")
            nc.scalar.dma_start(t[:], ginT_d[k0 : k0 + ksz, :])
            ginT.append(t)

        # ---- u = pw1e.T @ ejkT  (pair part of elem layer 1) ------------
        uT = [wp.tile([128, R], F32, tag=f"uT{m}", name=f"uT{m}") for m in range(2)]
        for m in range(2):
            for rc in range(2):
                psu = psS.tile([128, 512], F32, tag="small", name="psu")
                nc.tensor.matmul(
                    psu[:],
                    pw1e[m],
                    ejkT[:, rc * 512 : (rc + 1) * 512],
                    start=True,
                    stop=True,
                )
                nc.vector.tensor_copy(
                    uT[m][:, rc * 512 : (rc + 1) * 512], psu[:]
                )

        # first-iteration x1, emitted before everything else on ScalarE
        x1_first = []
        for m in range(2):
            o = ap.tile([128, R], F32R, tag=f"x1_{m}", name=f"x1_{m}")
            nc.scalar.activation(o[:], uT[m][:], AF.Silu, bias=vb1[m][:, 0:1])
            x1_first.append(o)

        # ---- geometry MLP (emission staged into the ne loop below) -----
        gm_ctx = {}

        def mlp_half(rhs_tiles, weights, bias_col, out_name):
            ps = psB.tile([128, R], F32, tag="big", name="ps_mlp")
            nk = len(rhs_tiles)
            for rc in range(2):
                for k in range(nk):
                    nc.tensor.matmul(
                        ps[:, rc * 512 : (rc + 1) * 512],
                        weights[k],
                        rhs_tiles[k][:, rc * 512 : (rc + 1) * 512],
                        start=(k == 0),
                        stop=(k == nk - 1),
                    )
            o = wp.tile([128, R], F32R, tag=out_name, name=out_name)
            nc.scalar.activation(
                o[:], ps[:], AF.Silu, bias=bia[:, bias_col : bias_col + 1]
            )
            return o

        def gm_stage_0():
            gm_ctx["x1g0"] = mlp_half(ginT, [w[0] for w in gmW1], GM_B1, "gx1_0")

        def gm_stage_1():
            gm_ctx["x1g1"] = mlp_half(ginT, [w[1] for w in gmW1], GM_B1 + 1, "gx1_1")

        def gm_stage_2():
            x1g = [gm_ctx["x1g0"], gm_ctx["x1g1"]]
            gm_ctx["x2g0"] = mlp_half(x1g, [w[0] for w in gmW2], GM_B2, "gx2_0")

        def gm_stage_3():
            x1g = [gm_ctx["x1g0"], gm_ctx["x1g1"]]
            gm_ctx["x2g1"] = mlp_half(x1g, [w[1] for w in gmW2], GM_B2 + 1, "gx2_1")

        def gm_stage_4():
            x2g = [gm_ctx["x2g0"], gm_ctx["x2g1"]]
            psG = psS.tile([64, R], F32, tag="small", name="psG")
            for rc in range(2):
                for k in range(2):
                    nc.tensor.matmul(
                        psG[:, rc * 512 : (rc + 1) * 512],
                        gmW3[k],
                        x2g[k][:, rc * 512 : (rc + 1) * 512],
                        start=(k == 0),
                        stop=(k == 1),
                    )
            Gtmp = sp.tile([64, R], F32, tag="Gtmp", name="Gtmp")
            nc.vector.tensor_scalar_add(Gtmp[:], psG[:], bia[0:64, GM_B3 : GM_B3 + 1])
            Gm = wp.tile([64, R], F32, tag="Gm", name="Gm")
            nc.vector.tensor_mul(Gm[:], Gtmp[:], pmask2)
            gm_ctx["Gm"] = Gm

        gm_stages = [gm_stage_0, gm_stage_1, gm_stage_2, gm_stage_3, gm_stage_4]

        # ---- final output MLP, one 128-column half at a time -----------
        aggT = wp.tile([64, NE * BPC], F32, tag="aggT")
        out_sb = [sp.tile([128, 256], F32, tag=f"oT_{m}", name=f"oT_{m}") for m in range(2)]

        def final_half(hc):
            cs = slice(hc * 64, (hc + 1) * 64)
            aggR = sp.tile([64, 64], F32R, tag="aggR", name="aggR", bufs=4)
            nc.vector.tensor_copy(aggR[:], aggT[:, cs])
            f1 = []
            for m in range(2):
                ps = psS.tile([128, 64], F32, tag="small", name="psf1")
                nc.tensor.matmul(ps[:], opW1[m], aggR[:], start=True, stop=True)
                o = sp.tile([128, 64], F32R, tag=f"f1_{m}", name=f"f1_{m}")
                nc.scalar.activation(
                    o[:], ps[:], AF.Silu, bias=bia[:, OP_B1 + m : OP_B1 + m + 1]
                )
                f1.append(o)
            for m in range(2):
                ps = psS.tile([128, 64], F32, tag="small", name="psf2")
                for k in range(2):
                    nc.tensor.matmul(
                        ps[:], opW2[k][m], f1[k][:], start=(k == 0), stop=(k == 1)
                    )
                nc.vector.tensor_scalar_add(
                    out_sb[m][:, cs], ps[:], bia[:, OP_B2 + m : OP_B2 + m + 1]
                )
                nc.sync.dma_start(out_d[m, :, cs], out_sb[m][:, cs])

        # ---- the ne loop (x1 prefetched one iteration ahead) -----------
        def emit_x1(ne):
            x1 = []
            for m in range(2):
                o = ap.tile([128, R], F32R, tag=f"x1_{m}", name=f"x1_{m}")
                nc.scalar.activation(
                    o[:], uT[m][:], AF.Silu, bias=vb1[m][:, ne : ne + 1]
                )
                x1.append(o)
            return x1

        def emit_amr(ne, g3sb):
            Gm = gm_ctx["Gm"]
            for b in range(BPC):
                scr = sp.tile([64, PMAX], F32, tag="scr", name="scr")
                nc.vector.affine_mul_reduce(
                    out=scr[:],
                    accum_out=aggT[:, ne * BPC + b : ne * BPC + b + 1],
                    in0=g3sb[:, b * PMAX : (b + 1) * PMAX],
                    in1=Gm[:, b * PMAX : (b + 1) * PMAX],
                    scale=1.0,
                    bias=bia[0:64, PE_B3 : PE_B3 + 1],
                )

        pending = []
        x1 = x1_first
        for ne in range(NE):
            x1_next = emit_x1(ne + 1) if ne + 1 < NE else None
            x2 = []
            for m in range(2):
                ps = psB.tile([128, R], F32, tag="big", name="ps2")
                for rc in range(2):
                    for k in range(2):
                        nc.tensor.matmul(
                            ps[:, rc * 512 : (rc + 1) * 512],
                            peW2[k][m],
                            x1[k][:, rc * 512 : (rc + 1) * 512],
                            start=(k == 0),
                            stop=(k == 1),
                        )
                o = ap.tile([128, R], F32R, tag=f"x2_{m}", name=f"x2_{m}")
                nc.scalar.activation(
                    o[:], ps[:], AF.Silu, bias=bia[:, PE_B2 + m : PE_B2 + m + 1]
                )
                x2.append(o)
            ps3 = psS.tile([64, R], F32, tag="small", name="ps3")
            for rc in range(2):
                for k in range(2):
                    nc.tensor.matmul(
                        ps3[:, rc * 512 : (rc + 1) * 512],
                        peW3[k],
                        x2[k][:, rc * 512 : (rc + 1) * 512],
                        start=(k == 0),
                        stop=(k == 1),
                    )
            g3sb = ap.tile([64, R], F16, tag="g3sb", name="g3sb", bufs=24)
            nc.vector.tensor_copy(g3sb[:], ps3[:])
            if "Gm" in gm_ctx:
                for pne, pg in pending:
                    emit_amr(pne, pg)
                pending.clear()
                emit_amr(ne, g3sb)
            else:
                pending.append((ne, g3sb))
            if ne >= 6 and (ne - 6) % 3 == 0 and (ne - 6) // 3 < len(gm_stages):
                gm_stages[(ne - 6) // 3]()
            x1 = x1_next
            if ne == 19:
                final_half(0)
            elif ne == 31:
                final_half(1)
            elif ne == 47:
                final_half(2)
        final_half(3)

    nc.compile()
    return nc


def _get_nc():
    if "nc" not in _CACHE:
        _CACHE["nc"] = _build()
    return _CACHE["nc"]


# --------------------------------------------------------------------------
# entry point
# --------------------------------------------------------------------------

def kernel(
    h,
    z,
    pos,
    mask,
    e_feat,
    z_emb,
    gm_W1,
    gm_b1,
    gm_W2,
    gm_b2,
    gm_W3,
    gm_b3,
    pe_W1,
    pe_b1,
    pe_W2,
    pe_b2,
    pe_W3,
    pe_b3,
    op_W1,
    op_b1,
    op_W2,
    op_b2,
    absorber_index=0,
):
    h = np.asarray(h, np.float32)
    z = np.asarray(z).astype(np.int64)
    pos = np.asarray(pos, np.float32)
    mask = np.asarray(mask).astype(bool)
    e_feat = np.asarray(e_feat, np.float32)
    z_emb = np.asarray(z_emb, np.float32)
    gm_W1 = np.asarray(gm_W1, np.float32)
    gm_b1 = np.asarray(gm_b1, np.float32)
    gm_W2 = np.asarray(gm_W2, np.float32)
    gm_b2 = np.asarray(gm_b2, np.float32)
    gm_W3 = np.asarray(gm_W3, np.float32)
    gm_b3 = np.asarray(gm_b3, np.float32)
    pe_W1 = np.asarray(pe_W1, np.float32)
    pe_b1 = np.asarray(pe_b1, np.float32)
    pe_W2 = np.asarray(pe_W2, np.float32)
    pe_b2 = np.asarray(pe_b2, np.float32)
    pe_W3 = np.asarray(pe_W3, np.float32)
    pe_b3 = np.asarray(pe_b3, np.float32)
    op_W1 = np.asarray(op_W1, np.float32)
    op_b1 = np.asarray(op_b1, np.float32)
    op_W2 = np.asarray(op_W2, np.float32)
    op_b2 = np.asarray(op_b2, np.float32)

    geom_in, ejk, pmask = _host_prep(h, z, pos, mask, z_emb, absorber_index)

    # v[ne] = e_feat @ pe_W1[64:] + pe_b1, the ne-dependent layer-1 bias
    vb1_full = (e_feat @ pe_W1[2 * ZEMB :] + pe_b1).astype(np.float32)  # [NE, PH]
    vb1 = vb1_full.T.reshape(2, 128, NE)  # [mh, 128, NE]

    biases = np.zeros((128, NBIAS), np.float32)
    biases[:, GM_B1] = gm_b1[:128]
    biases[:, GM_B1 + 1] = gm_b1[128:]
    biases[:, GM_B2] = gm_b2[:128]
    biases[:, GM_B2 + 1] = gm_b2[128:]
    biases[0:64, GM_B3] = gm_b3
    biases[:, PE_B2] = pe_b2[:128]
    biases[:, PE_B2 + 1] = pe_b2[128:]
    biases[0:64, PE_B3] = pe_b3
    biases[:, OP_B1] = op_b1[:128]
    biases[:, OP_B1 + 1] = op_b1[128:]
    biases[:, OP_B2] = op_b2[:128]
    biases[:, OP_B2 + 1] = op_b2[128:]

    wpack = np.zeros((128, WCOLS), np.float32)

    def put(c0, w, msplit=True):  # w: [K, M], tiles of [<=128, 128-cols]
        kk, mm = w.shape
        for k in range(0, kk, 128):
            ksz = min(128, kk - k)
            for m in range(0, mm, 128):
                msz = min(128, mm - m)
                col = c0 + (k // 128) * mm + m
                wpack[0:ksz, col : col + msz] = w[k : k + ksz, m : m + msz]

    put(W_PEW2, pe_W2)
    put(W_PEW3, pe_W3)
    put(W_GMW1, gm_W1)
    put(W_GMW2, gm_W2)
    put(W_GMW3, gm_W3)
    put(W_OPW1, op_W1)
    put(W_OPW2, op_W2)

    in_maps = []
    for c in range(N_CORES):
        sl = slice(c * BPC, (c + 1) * BPC)
        gi = geom_in[sl]  # [BPC, P, 353]
        ginT = np.ascontiguousarray(gi.reshape(R, GIN).T)  # [353, R]
        dpack = np.zeros((128, DCOLS), np.float32)
        dpack[0:64, 0:R] = ejk[sl].reshape(R, 2 * ZEMB).T
        pm2 = np.broadcast_to(
            pmask[sl].reshape(1, R).astype(np.float32), (64, R)
        )
        dpack[0:64, D_PW1 : D_PW1 + PH] = pe_W1[: 2 * ZEMB]
        dpack[:, D_VB1 : D_VB1 + NE] = vb1[0]
        dpack[:, D_VB1 + NE : D_VB1 + 2 * NE] = vb1[1]
        dpack[:, D_BIA : D_BIA + NBIAS] = biases
        in_maps.append({"ginT": ginT, "dpack": dpack, "wpack": wpack, "pmask": np.ascontiguousarray(pm2)})

    nc = _get_nc()
    res = run_bass_kernel_spmd(nc, in_maps, list(range(N_CORES)), trace=TRACE)
    _CACHE["last_result"] = res

    out = np.empty((B, NE, OUT), np.float32)
    for c in range(N_CORES):
        oc = res.results[c]["out"]  # [2, 128, 256] = (mh, o, ne*BPC+b)
        oc = oc.reshape(OUT, NE, BPC)  # [256, 64, 4]
        out[c * BPC : (c + 1) * BPC] = oc.transpose(2, 1, 0)
    return out


# revision 14
# speedup vs baseline: 1.0377x; 1.0377x over previous
"""Trainium2 Bass kernel for nn_AbsorberPathAggregator.

Contract: kernel(**inputs) takes the FULL unsharded inputs (as produced by
setup_inputs()) and returns the FULL [B, NE, OUT] float32 output.

Strategy (pure data parallel over B across 8 NeuronCores, 4 samples/core):
  - Host (numpy): pair enumeration (argsort of valid triu pairs), gathers,
    RBF features -> geom_inT [353, 1024] per core; ejkT [64, 1024]; pmask.
    This is cheap index/bookkeeping work; all FLOP-heavy MLPs run on device.
  - Device per core, all activations kept transposed [feature, row]:
      G = gm-MLP(geom_inT) masked by pmask              (geometry path)
      u = pe_W1[:64].T @ ejkT                           (pair part of layer 1)
      for ne in 0..63:  x1 = Silu(u + v[ne]) ; x2 = Silu(W2.T x1)
                        g3 = W3.T x2 ; agg[:, ne*4+b] = sum_p (g3+b3)*G
      out = (Silu(opW1.T agg + b1)).T opW2 + b2  -> DMA out
    Matmuls use float32r (full-rate fp32-reduced), Silu on ScalarE with the
    free per-partition bias operand, epilogue via the fused affine_mul_reduce
    DVE op.
"""

import sys

for _p in (
    "/root/.axon_site",
    "/root/.axon_site/_ro/trn_rl_repo",
    "/root/.axon_site/_ro/pypackages",
    "/opt/trn_rl_repo",
):
    if _p not in sys.path:
        sys.path.append(_p)

import numpy as np

from contextlib import ExitStack

import concourse.tile as tile
from concourse import bacc, mybir
from concourse.bass_utils import run_bass_kernel_spmd

F32 = mybir.dt.float32
F32R = mybir.dt.float32r
F16 = mybir.dt.float16
AF = mybir.ActivationFunctionType
ALU = mybir.AluOpType

B, N, H = 32, 64, 128
RBF_DIM, GH, SCATTER, OUT = 32, 256, 64, 256
CUTOFF, PMAX = 6.0, 256
ZEMB, EDIM, NE, PH = 32, 32, 64, 256
GIN = 2 * H + 3 * RBF_DIM + 1  # 353

N_CORES = 8
BPC = B // N_CORES  # 4 samples per core
R = BPC * PMAX  # 1024 rows per core

TRACE = False  # set by test harness for NTFF profiling
_CACHE = {}


# --------------------------------------------------------------------------
# host-side preprocessing (numpy, mirrors the reference's indexing exactly)
# --------------------------------------------------------------------------

def _rbf(x):
    centers = np.linspace(0.0, CUTOFF, RBF_DIM, dtype=np.float32)
    delta = CUTOFF / (RBF_DIM - 1)
    gamma = 1.0 / (delta * delta + 1e-12)
    d = x[..., None] - centers
    return np.exp((-gamma) * d * d).astype(np.float32)


def _norm(v):
    return np.sqrt((v * v).sum(-1) + np.float32(1e-12))


def _host_prep(h, z, pos, mask, z_emb, absorber_index):
    ai = int(absorber_index)
    pos0 = pos[:, ai][:, None, :]  # [B,1,3]
    r = _norm(pos - pos0)  # [B,N]
    valid = mask & (np.arange(N)[None, :] != ai) & (r <= np.float32(CUTOFF))
    ju, ku = np.triu_indices(N, k=1)
    pv = valid[:, ju] & valid[:, ku]  # [B, Np]
    order = np.argsort(~pv, axis=1, kind="stable")[:, :PMAX]
    pmask = np.take_along_axis(pv, order, axis=1)  # [B,PMAX]
    j_idx = np.where(pmask, ju[order], 0)
    k_idx = np.where(pmask, ku[order], 0)
    bidx = np.arange(B)[:, None]
    hj, hk = h[bidx, j_idx], h[bidx, k_idx]  # [B,P,H]
    posj, posk = pos[bidx, j_idx], pos[bidx, k_idx]  # [B,P,3]
    vj, vk, vjk = posj - pos0, posk - pos0, posk - posj
    r0j, r0k, rjk = _norm(vj), _norm(vk), _norm(vjk)
    uj = vj / np.maximum(r0j[..., None], np.float32(1e-8))
    uk = vk / np.maximum(r0k[..., None], np.float32(1e-8))
    cosang = np.clip((uj * uk).sum(-1, keepdims=True), -1.0, 1.0).astype(np.float32)
    geom_in = np.concatenate(
        [
            hj,
            hk,
            _rbf(np.minimum(r0j, np.float32(CUTOFF))),
            _rbf(np.minimum(r0k, np.float32(CUTOFF))),
            _rbf(np.minimum(rjk, np.float32(CUTOFF))),
            cosang,
        ],
        axis=-1,
    ).astype(np.float32)  # [B, P, 353]
    ejk = np.concatenate([z_emb[z[bidx, j_idx]], z_emb[z[bidx, k_idx]]], axis=-1)
    return geom_in, ejk.astype(np.float32), pmask


# --------------------------------------------------------------------------
# device kernel
# --------------------------------------------------------------------------

# bias-pack column indices (columns within the dpack bias block)
GM_B1, GM_B2, GM_B3, PE_B2, PE_B3, OP_B1, OP_B2 = 0, 2, 4, 5, 7, 8, 10
NBIAS = 12

# dpack [128, DCOLS]: rows 0:64 cols 0:R = ejkT; rows 64:128 cols 0:R = pmask
# broadcast; rows 0:64 cols R:R+256 = pe_W1[:64]; cols R+256:R+384 = vb1 (2 mh
# halves of 64); cols R+384 : R+384+NBIAS = bias pack.
D_PW1 = R
D_VB1 = R + 256
D_BIA = R + 384
DCOLS = R + 384 + NBIAS

# wpack [128, WCOLS] column layout: peW2(512) peW3(128) gmW1(768) gmW2(512)
# gmW3(128) opW1(256) opW2(512)
W_PEW2 = 0
W_PEW3 = 512
W_GMW1 = 640
W_GMW2 = 1408
W_GMW3 = 1920
W_OPW1 = 2048
W_OPW2 = 2304
WCOLS = 2816
W_SPLIT = 640  # first chunk (pe weights) loaded on sync, rest on gpsimd


def _build():
    nc = bacc.Bacc("TRN2", target_bir_lowering=False, debug=False, num_devices=N_CORES)

    ginT_d = nc.dram_tensor("ginT", [GIN, R], F32R, kind="ExternalInput").ap()
    pmask_d = nc.dram_tensor("pmask", [64, R], F32, kind="ExternalInput").ap()
    dpack_d = nc.dram_tensor("dpack", [128, DCOLS], F32R, kind="ExternalInput").ap()
    wpack_d = nc.dram_tensor("wpack", [128, WCOLS], F32R, kind="ExternalInput").ap()
    out_d = nc.dram_tensor("out", [2, 128, 256], F32, kind="ExternalOutput").ap()

    K1 = [(0, 128), (128, 128), (256, GIN - 256)]  # gm layer-1 k tiles

    with tile.TileContext(nc) as tc, ExitStack() as ctx:
        wp = ctx.enter_context(tc.tile_pool(name="wp", bufs=1))
        ap = ctx.enter_context(tc.tile_pool(name="ap", bufs=3))
        sp = ctx.enter_context(tc.tile_pool(name="sp", bufs=2))
        psB = ctx.enter_context(tc.tile_pool(name="psB", bufs=2, space="PSUM"))
        psS = ctx.enter_context(tc.tile_pool(name="psS", bufs=2, space="PSUM"))

        # ---- loads (spread across engine queues; dpack is critical) ----
        dpack = wp.tile([128, DCOLS], F32R, tag="dpack")
        nc.sync.dma_start(dpack[0:64, 0:R], dpack_d[0:64, 0:R])
        nc.sync.dma_start(dpack[:, R:DCOLS], dpack_d[:, R:DCOLS])
        wpack = wp.tile([128, WCOLS], F32R, tag="wpack")
        nc.sync.dma_start(wpack[:, 0:W_SPLIT], wpack_d[:, 0:W_SPLIT])
        nc.sync.dma_start(wpack[:, W_SPLIT:W_GMW2], wpack_d[:, W_SPLIT:W_GMW2])
        nc.sync.dma_start(wpack[:, W_GMW2:], wpack_d[:, W_GMW2:])
        pmask2t = wp.tile([64, R], F32, tag="pmask2t")
        nc.sync.dma_start(pmask2t[:], pmask_d)
        pmask2 = pmask2t[:]

        ejkT = dpack[0:64, 0:R]
        pw1e = [dpack[0:64, D_PW1 + m * 128 : D_PW1 + (m + 1) * 128] for m in range(2)]
        vb1 = [
            dpack[:, D_VB1 + m * NE : D_VB1 + (m + 1) * NE].bitcast(F32)
            for m in range(2)
        ]
        bia = dpack[:, D_BIA : D_BIA + NBIAS].bitcast(F32)

        def wtile(c0, ksz, m):
            return wpack[0:ksz, c0 + m * 128 : c0 + (m + 1) * 128]

        peW2 = [[wtile(W_PEW2 + k * 256, 128, m) for m in range(2)] for k in range(2)]
        peW3 = [wpack[0:128, W_PEW3 + k * 64 : W_PEW3 + (k + 1) * 64] for k in range(2)]
        gmW1 = [
            [wtile(W_GMW1 + k * 256, K1[k][1], m) for m in range(2)] for k in range(3)
        ]
        gmW2 = [[wtile(W_GMW2 + k * 256, 128, m) for m in range(2)] for k in range(2)]
        gmW3 = [wpack[0:128, W_GMW3 + k * 64 : W_GMW3 + (k + 1) * 64] for k in range(2)]
        opW1 = [wpack[0:64, W_OPW1 + m * 128 : W_OPW1 + (m + 1) * 128] for m in range(2)]
        opW2 = [[wtile(W_OPW2 + k * 256, 128, m) for m in range(2)] for k in range(2)]

        ginT = []
        for k, (k0, ksz) in enumerate(K1):
            t = wp.tile([ksz, R], F32R, tag=f"ginT{k}", name=f"ginT{k}")
            nc.scalar.dma_start(t[:], ginT_d[k0 : k0 + ksz, :])
            ginT.append(t)

        # ---- u = pw1e.T @ ejkT  (pair part of elem layer 1) ------------
        uT = [wp.tile([128, R], F32, tag=f"uT{m}", name=f"uT{m}") for m in range(2)]
        for m in range(2):
            for rc in range(2):
                psu = psS.tile([128, 512], F32, tag="small", name="psu")
                nc.tensor.matmul(
                    psu[:],
                    pw1e[m],
                    ejkT[:, rc * 512 : (rc + 1) * 512],
                    start=True,
                    stop=True,
                )
                nc.vector.tensor_copy(
                    uT[m][:, rc * 512 : (rc + 1) * 512], psu[:]
                )

        # first-iteration x1, emitted before everything else on ScalarE
        x1_first = []
        for m in range(2):
            o = ap.tile([128, R], F32R, tag=f"x1_{m}", name=f"x1_{m}")
            nc.scalar.activation(o[:], uT[m][:], AF.Silu, bias=vb1[m][:, 0:1])
            x1_first.append(o)

        # ---- geometry MLP (emission staged into the ne loop below) -----
        gm_ctx = {}

        def mlp_half(rhs_tiles, weights, bias_col, out_name):
            ps = psB.tile([128, R], F32, tag="big", name="ps_mlp")
            nk = len(rhs_tiles)
            for rc in range(2):
                for k in range(nk):
                    nc.tensor.matmul(
                        ps[:, rc * 512 : (rc + 1) * 512],
                        weights[k],
                        rhs_tiles[k][:, rc * 512 : (rc + 1) * 512],
                        start=(k == 0),
                        stop=(k == nk - 1),
                    )
            o = wp.tile([128, R], F32R, tag=out_name, name=out_name)
            nc.scalar.activation(
                o[:], ps[:], AF.Silu, bias=bia[:, bias_col : bias_col + 1]
            )
            return o

        def gm_stage_0():
            gm_ctx["x1g0"] = mlp_half(ginT, [w[0] for w in gmW1], GM_B1, "gx1_0")

        def gm_stage_1():
            gm_ctx["x1g1"] = mlp_half(ginT, [w[1] for w in gmW1], GM_B1 + 1, "gx1_1")

        def gm_stage_2():
            x1g = [gm_ctx["x1g0"], gm_ctx["x1g1"]]
            gm_ctx["x2g0"] = mlp_half(x1g, [w[0] for w in gmW2], GM_B2, "gx2_0")

        def gm_stage_3():
            x1g = [gm_ctx["x1g0"], gm_ctx["x1g1"]]
            gm_ctx["x2g1"] = mlp_half(x1g, [w[1] for w in gmW2], GM_B2 + 1, "gx2_1")

        def gm_stage_4():
            x2g = [gm_ctx["x2g0"], gm_ctx["x2g1"]]
            psG = psS.tile([64, R], F32, tag="small", name="psG")
            for rc in range(2):
                for k in range(2):
                    nc.tensor.matmul(
                        psG[:, rc * 512 : (rc + 1) * 512],
                        gmW3[k],
                        x2g[k][:, rc * 512 : (rc + 1) * 512],
                        start=(k == 0),
                        stop=(k == 1),
                    )
            Gtmp = sp.tile([64, R], F32, tag="Gtmp", name="Gtmp")
            nc.vector.tensor_scalar_add(Gtmp[:], psG[:], bia[0:64, GM_B3 : GM_B3 + 1])
            Gm = wp.tile([64, R], F32, tag="Gm", name="Gm")
            nc.vector.tensor_mul(Gm[:], Gtmp[:], pmask2)
            gm_ctx["Gm"] = Gm

        gm_stages = [gm_stage_0, gm_stage_1, gm_stage_2, gm_stage_3, gm_stage_4]

        # ---- final output MLP, one 128-column half at a time -----------
        aggT = wp.tile([64, NE * BPC], F32, tag="aggT")
        out_sb = [sp.tile([128, 256], F32, tag=f"oT_{m}", name=f"oT_{m}") for m in range(2)]

        def final_half(hc):
            cs = slice(hc * 64, (hc + 1) * 64)
            aggR = sp.tile([64, 64], F32R, tag="aggR", name="aggR", bufs=4)
            nc.vector.tensor_copy(aggR[:], aggT[:, cs])
            f1 = []
            for m in range(2):
                ps = psS.tile([128, 64], F32, tag="small", name="psf1")
                nc.tensor.matmul(ps[:], opW1[m], aggR[:], start=True, stop=True)
                o = sp.tile([128, 64], F32R, tag=f"f1_{m}", name=f"f1_{m}")
                nc.scalar.activation(
                    o[:], ps[:], AF.Silu, bias=bia[:, OP_B1 + m : OP_B1 + m + 1]
                )
                f1.append(o)
            for m in range(2):
                ps = psS.tile([128, 64], F32, tag="small", name="psf2")
                for k in range(2):
                    nc.tensor.matmul(
                        ps[:], opW2[k][m], f1[k][:], start=(k == 0), stop=(k == 1)
                    )
                nc.vector.tensor_scalar_add(
                    out_sb[m][:, cs], ps[:], bia[:, OP_B2 + m : OP_B2 + m + 1]
                )
                nc.sync.dma_start(out_d[m, :, cs], out_sb[m][:, cs])

        # ---- the ne loop (x1 prefetched one iteration ahead) -----------
        def emit_x1(ne):
            x1 = []
            for m in range(2):
                o = ap.tile([128, R], F32R, tag=f"x1_{m}", name=f"x1_{m}")
                nc.scalar.activation(
                    o[:], uT[m][:], AF.Silu, bias=vb1[m][:, ne : ne + 1]
                )
                x1.append(o)
            return x1

        def emit_amr(ne, g3sb):
            Gm = gm_ctx["Gm"]
            for b in range(BPC):
                scr = sp.tile([64, PMAX], F32, tag="scr", name="scr")
                nc.vector.affine_mul_reduce(
                    out=scr[:],
                    accum_out=aggT[:, ne * BPC + b : ne * BPC + b + 1],
                    in0=g3sb[:, b * PMAX : (b + 1) * PMAX],
                    in1=Gm[:, b * PMAX : (b + 1) * PMAX],
                    scale=1.0,
                    bias=bia[0:64, PE_B3 : PE_B3 + 1],
                )

        pending = []
        x1 = x1_first
        for ne in range(NE):
            x1_next = emit_x1(ne + 1) if ne + 1 < NE else None
            x2 = []
            for m in range(2):
                ps = psB.tile([128, R], F32, tag="big", name="ps2")
                for rc in range(2):
                    for k in range(2):
                        nc.tensor.matmul(
                            ps[:, rc * 512 : (rc + 1) * 512],
                            peW2[k][m],
                            x1[k][:, rc * 512 : (rc + 1) * 512],
                            start=(k == 0),
                            stop=(k == 1),
                        )
                o = ap.tile([128, R], F32R, tag=f"x2_{m}", name=f"x2_{m}")
                nc.scalar.activation(
                    o[:], ps[:], AF.Silu, bias=bia[:, PE_B2 + m : PE_B2 + m + 1]
                )
                x2.append(o)
            ps3 = psS.tile([64, R], F32, tag="small", name="ps3")
            for rc in range(2):
                for k in range(2):
                    nc.tensor.matmul(
                        ps3[:, rc * 512 : (rc + 1) * 512],
                        peW3[k],
                        x2[k][:, rc * 512 : (rc + 1) * 512],
                        start=(k == 0),
                        stop=(k == 1),
                    )
            g3sb = ap.tile([64, R], F16, tag="g3sb", name="g3sb", bufs=24)
            nc.vector.tensor_copy(g3sb[:], ps3[:])
            if "Gm" in gm_ctx:
                for pne, pg in pending[:4]:
                    emit_amr(pne, pg)
                del pending[:4]
                emit_amr(ne, g3sb)
            else:
                pending.append((ne, g3sb))
            if ne >= 6 and (ne - 6) % 2 == 0 and (ne - 6) // 2 < len(gm_stages):
                gm_stages[(ne - 6) // 2]()
            x1 = x1_next
            if ne == 22:
                final_half(0)
            elif ne == 33:
                final_half(1)
            elif ne == 48:
                final_half(2)
        final_half(3)

    nc.compile()
    return nc


def _get_nc():
    if "nc" not in _CACHE:
        _CACHE["nc"] = _build()
    return _CACHE["nc"]


# --------------------------------------------------------------------------
# entry point
# --------------------------------------------------------------------------

def kernel(
    h,
    z,
    pos,
    mask,
    e_feat,
    z_emb,
    gm_W1,
    gm_b1,
    gm_W2,
    gm_b2,
    gm_W3,
    gm_b3,
    pe_W1,
    pe_b1,
    pe_W2,
    pe_b2,
    pe_W3,
    pe_b3,
    op_W1,
    op_b1,
    op_W2,
    op_b2,
    absorber_index=0,
):
    h = np.asarray(h, np.float32)
    z = np.asarray(z).astype(np.int64)
    pos = np.asarray(pos, np.float32)
    mask = np.asarray(mask).astype(bool)
    e_feat = np.asarray(e_feat, np.float32)
    z_emb = np.asarray(z_emb, np.float32)
    gm_W1 = np.asarray(gm_W1, np.float32)
    gm_b1 = np.asarray(gm_b1, np.float32)
    gm_W2 = np.asarray(gm_W2, np.float32)
    gm_b2 = np.asarray(gm_b2, np.float32)
    gm_W3 = np.asarray(gm_W3, np.float32)
    gm_b3 = np.asarray(gm_b3, np.float32)
    pe_W1 = np.asarray(pe_W1, np.float32)
    pe_b1 = np.asarray(pe_b1, np.float32)
    pe_W2 = np.asarray(pe_W2, np.float32)
    pe_b2 = np.asarray(pe_b2, np.float32)
    pe_W3 = np.asarray(pe_W3, np.float32)
    pe_b3 = np.asarray(pe_b3, np.float32)
    op_W1 = np.asarray(op_W1, np.float32)
    op_b1 = np.asarray(op_b1, np.float32)
    op_W2 = np.asarray(op_W2, np.float32)
    op_b2 = np.asarray(op_b2, np.float32)

    geom_in, ejk, pmask = _host_prep(h, z, pos, mask, z_emb, absorber_index)

    # v[ne] = e_feat @ pe_W1[64:] + pe_b1, the ne-dependent layer-1 bias
    vb1_full = (e_feat @ pe_W1[2 * ZEMB :] + pe_b1).astype(np.float32)  # [NE, PH]
    vb1 = vb1_full.T.reshape(2, 128, NE)  # [mh, 128, NE]

    biases = np.zeros((128, NBIAS), np.float32)
    biases[:, GM_B1] = gm_b1[:128]
    biases[:, GM_B1 + 1] = gm_b1[128:]
    biases[:, GM_B2] = gm_b2[:128]
    biases[:, GM_B2 + 1] = gm_b2[128:]
    biases[0:64, GM_B3] = gm_b3
    biases[:, PE_B2] = pe_b2[:128]
    biases[:, PE_B2 + 1] = pe_b2[128:]
    biases[0:64, PE_B3] = pe_b3
    biases[:, OP_B1] = op_b1[:128]
    biases[:, OP_B1 + 1] = op_b1[128:]
    biases[:, OP_B2] = op_b2[:128]
    biases[:, OP_B2 + 1] = op_b2[128:]

    wpack = np.zeros((128, WCOLS), np.float32)

    def put(c0, w, msplit=True):  # w: [K, M], tiles of [<=128, 128-cols]
        kk, mm = w.shape
        for k in range(0, kk, 128):
            ksz = min(128, kk - k)
            for m in range(0, mm, 128):
                msz = min(128, mm - m)
                col = c0 + (k // 128) * mm + m
                wpack[0:ksz, col : col + msz] = w[k : k + ksz, m : m + msz]

    put(W_PEW2, pe_W2)
    put(W_PEW3, pe_W3)
    put(W_GMW1, gm_W1)
    put(W_GMW2, gm_W2)
    put(W_GMW3, gm_W3)
    put(W_OPW1, op_W1)
    put(W_OPW2, op_W2)

    in_maps = []
    for c in range(N_CORES):
        sl = slice(c * BPC, (c + 1) * BPC)
        gi = geom_in[sl]  # [BPC, P, 353]
        ginT = np.ascontiguousarray(gi.reshape(R, GIN).T)  # [353, R]
        dpack = np.zeros((128, DCOLS), np.float32)
        dpack[0:64, 0:R] = ejk[sl].reshape(R, 2 * ZEMB).T
        pm2 = np.broadcast_to(
            pmask[sl].reshape(1, R).astype(np.float32), (64, R)
        )
        dpack[0:64, D_PW1 : D_PW1 + PH] = pe_W1[: 2 * ZEMB]
        dpack[:, D_VB1 : D_VB1 + NE] = vb1[0]
        dpack[:, D_VB1 + NE : D_VB1 + 2 * NE] = vb1[1]
        dpack[:, D_BIA : D_BIA + NBIAS] = biases
        in_maps.append({"ginT": ginT, "dpack": dpack, "wpack": wpack, "pmask": np.ascontiguousarray(pm2)})

    nc = _get_nc()
    res = run_bass_kernel_spmd(nc, in_maps, list(range(N_CORES)), trace=TRACE)
    _CACHE["last_result"] = res

    out = np.empty((B, NE, OUT), np.float32)
    for c in range(N_CORES):
        oc = res.results[c]["out"]  # [2, 128, 256] = (mh, o, ne*BPC+b)
        oc = oc.reshape(OUT, NE, BPC)  # [256, 64, 4]
        out[c * BPC : (c + 1) * BPC] = oc.transpose(2, 1, 0)
    return out


# revision 15
# speedup vs baseline: 1.0571x; 1.0187x over previous
"""Trainium2 Bass kernel for nn_AbsorberPathAggregator.

Contract: kernel(**inputs) takes the FULL unsharded inputs (as produced by
setup_inputs()) and returns the FULL [B, NE, OUT] float32 output.

Strategy (pure data parallel over B across 8 NeuronCores, 4 samples/core):
  - Host (numpy): pair enumeration (argsort of valid triu pairs), gathers,
    RBF features -> geom_inT [353, 1024] per core; ejkT [64, 1024]; pmask.
    This is cheap index/bookkeeping work; all FLOP-heavy MLPs run on device.
  - Device per core, all activations kept transposed [feature, row]:
      G = gm-MLP(geom_inT) masked by pmask              (geometry path)
      u = pe_W1[:64].T @ ejkT                           (pair part of layer 1)
      for ne in 0..63:  x1 = Silu(u + v[ne]) ; x2 = Silu(W2.T x1)
                        g3 = W3.T x2 ; agg[:, ne*4+b] = sum_p (g3+b3)*G
      out = (Silu(opW1.T agg + b1)).T opW2 + b2  -> DMA out
    Matmuls use float32r (full-rate fp32-reduced), Silu on ScalarE with the
    free per-partition bias operand, epilogue via the fused affine_mul_reduce
    DVE op.
"""

import sys

for _p in (
    "/root/.axon_site",
    "/root/.axon_site/_ro/trn_rl_repo",
    "/root/.axon_site/_ro/pypackages",
    "/opt/trn_rl_repo",
):
    if _p not in sys.path:
        sys.path.append(_p)

import numpy as np

from contextlib import ExitStack

import concourse.tile as tile
from concourse import bacc, mybir
from concourse.bass_utils import run_bass_kernel_spmd

F32 = mybir.dt.float32
F32R = mybir.dt.float32r
F16 = mybir.dt.float16
AF = mybir.ActivationFunctionType
ALU = mybir.AluOpType

B, N, H = 32, 64, 128
RBF_DIM, GH, SCATTER, OUT = 32, 256, 64, 256
CUTOFF, PMAX = 6.0, 256
ZEMB, EDIM, NE, PH = 32, 32, 64, 256
GIN = 2 * H + 3 * RBF_DIM + 1  # 353

N_CORES = 8
BPC = B // N_CORES  # 4 samples per core
R = BPC * PMAX  # 1024 rows per core

TRACE = False  # set by test harness for NTFF profiling
_CACHE = {}


# --------------------------------------------------------------------------
# host-side preprocessing (numpy, mirrors the reference's indexing exactly)
# --------------------------------------------------------------------------

def _rbf(x):
    centers = np.linspace(0.0, CUTOFF, RBF_DIM, dtype=np.float32)
    delta = CUTOFF / (RBF_DIM - 1)
    gamma = 1.0 / (delta * delta + 1e-12)
    d = x[..., None] - centers
    return np.exp((-gamma) * d * d).astype(np.float32)


def _norm(v):
    return np.sqrt((v * v).sum(-1) + np.float32(1e-12))


def _host_prep(h, z, pos, mask, z_emb, absorber_index):
    ai = int(absorber_index)
    pos0 = pos[:, ai][:, None, :]  # [B,1,3]
    r = _norm(pos - pos0)  # [B,N]
    valid = mask & (np.arange(N)[None, :] != ai) & (r <= np.float32(CUTOFF))
    ju, ku = np.triu_indices(N, k=1)
    pv = valid[:, ju] & valid[:, ku]  # [B, Np]
    order = np.argsort(~pv, axis=1, kind="stable")[:, :PMAX]
    pmask = np.take_along_axis(pv, order, axis=1)  # [B,PMAX]
    j_idx = np.where(pmask, ju[order], 0)
    k_idx = np.where(pmask, ku[order], 0)
    bidx = np.arange(B)[:, None]
    hj, hk = h[bidx, j_idx], h[bidx, k_idx]  # [B,P,H]
    posj, posk = pos[bidx, j_idx], pos[bidx, k_idx]  # [B,P,3]
    vj, vk, vjk = posj - pos0, posk - pos0, posk - posj
    r0j, r0k, rjk = _norm(vj), _norm(vk), _norm(vjk)
    uj = vj / np.maximum(r0j[..., None], np.float32(1e-8))
    uk = vk / np.maximum(r0k[..., None], np.float32(1e-8))
    cosang = np.clip((uj * uk).sum(-1, keepdims=True), -1.0, 1.0).astype(np.float32)
    geom_in = np.concatenate(
        [
            hj,
            hk,
            _rbf(np.minimum(r0j, np.float32(CUTOFF))),
            _rbf(np.minimum(r0k, np.float32(CUTOFF))),
            _rbf(np.minimum(rjk, np.float32(CUTOFF))),
            cosang,
        ],
        axis=-1,
    ).astype(np.float32)  # [B, P, 353]
    ejk = np.concatenate([z_emb[z[bidx, j_idx]], z_emb[z[bidx, k_idx]]], axis=-1)
    return geom_in, ejk.astype(np.float32), pmask


# --------------------------------------------------------------------------
# device kernel
# --------------------------------------------------------------------------

# bias-pack column indices (columns within the dpack bias block)
GM_B1, GM_B2, GM_B3, PE_B2, PE_B3, OP_B1, OP_B2 = 0, 2, 4, 5, 7, 8, 10
NBIAS = 12

# dpack [128, DCOLS]: rows 0:64 cols 0:R = ejkT; rows 64:128 cols 0:R = pmask
# broadcast; rows 0:64 cols R:R+256 = pe_W1[:64]; cols R+256:R+384 = vb1 (2 mh
# halves of 64); cols R+384 : R+384+NBIAS = bias pack.
D_PW1 = R
D_VB1 = R + 256
D_BIA = R + 384
DCOLS = R + 384 + NBIAS

# wpack [128, WCOLS] column layout: peW2(512) peW3(128) gmW1(768) gmW2(512)
# gmW3(128) opW1(256) opW2(512)
W_PEW2 = 0
W_PEW3 = 512
W_GMW1 = 640
W_GMW2 = 1408
W_GMW3 = 1920
W_OPW1 = 2048
W_OPW2 = 2304
WCOLS = 2816
W_SPLIT = 640  # first chunk (pe weights) loaded on sync, rest on gpsimd


def _build():
    nc = bacc.Bacc("TRN2", target_bir_lowering=False, debug=False, num_devices=N_CORES)

    ginT_d = nc.dram_tensor("ginT", [GIN, R], F32R, kind="ExternalInput").ap()
    pmask_d = nc.dram_tensor("pmask", [64, R], F32, kind="ExternalInput").ap()
    dpack_d = nc.dram_tensor("dpack", [128, DCOLS], F32R, kind="ExternalInput").ap()
    wpack_d = nc.dram_tensor("wpack", [128, WCOLS], F32R, kind="ExternalInput").ap()
    out_d = nc.dram_tensor("out", [2, 128, 256], F32, kind="ExternalOutput").ap()

    K1 = [(0, 128), (128, 128), (256, GIN - 256)]  # gm layer-1 k tiles

    with tile.TileContext(nc) as tc, ExitStack() as ctx:
        wp = ctx.enter_context(tc.tile_pool(name="wp", bufs=1))
        ap = ctx.enter_context(tc.tile_pool(name="ap", bufs=3))
        sp = ctx.enter_context(tc.tile_pool(name="sp", bufs=2))
        psB = ctx.enter_context(tc.tile_pool(name="psB", bufs=2, space="PSUM"))
        psS = ctx.enter_context(tc.tile_pool(name="psS", bufs=2, space="PSUM"))

        # ---- loads (spread across engine queues; dpack is critical) ----
        dpack = wp.tile([128, DCOLS], F32R, tag="dpack")
        nc.sync.dma_start(dpack[0:64, 0:R], dpack_d[0:64, 0:R])
        nc.sync.dma_start(dpack[:, R:DCOLS], dpack_d[:, R:DCOLS])
        wpack = wp.tile([128, WCOLS], F32R, tag="wpack")
        nc.sync.dma_start(wpack[:, 0:W_SPLIT], wpack_d[:, 0:W_SPLIT])
        nc.sync.dma_start(wpack[:, W_SPLIT:W_GMW2], wpack_d[:, W_SPLIT:W_GMW2])
        nc.sync.dma_start(wpack[:, W_GMW2:], wpack_d[:, W_GMW2:])
        pmask2t = wp.tile([64, R], F32, tag="pmask2t")
        nc.sync.dma_start(pmask2t[:], pmask_d)
        pmask2 = pmask2t[:]

        ejkT = dpack[0:64, 0:R]
        pw1e = [dpack[0:64, D_PW1 + m * 128 : D_PW1 + (m + 1) * 128] for m in range(2)]
        vb1 = [
            dpack[:, D_VB1 + m * NE : D_VB1 + (m + 1) * NE].bitcast(F32)
            for m in range(2)
        ]
        bia = dpack[:, D_BIA : D_BIA + NBIAS].bitcast(F32)

        def wtile(c0, ksz, m):
            return wpack[0:ksz, c0 + m * 128 : c0 + (m + 1) * 128]

        peW2 = [[wtile(W_PEW2 + k * 256, 128, m) for m in range(2)] for k in range(2)]
        peW3 = [wpack[0:128, W_PEW3 + k * 64 : W_PEW3 + (k + 1) * 64] for k in range(2)]
        gmW1 = [
            [wtile(W_GMW1 + k * 256, K1[k][1], m) for m in range(2)] for k in range(3)
        ]
        gmW2 = [[wtile(W_GMW2 + k * 256, 128, m) for m in range(2)] for k in range(2)]
        gmW3 = [wpack[0:128, W_GMW3 + k * 64 : W_GMW3 + (k + 1) * 64] for k in range(2)]
        opW1 = [wpack[0:64, W_OPW1 + m * 128 : W_OPW1 + (m + 1) * 128] for m in range(2)]
        opW2 = [[wtile(W_OPW2 + k * 256, 128, m) for m in range(2)] for k in range(2)]

        ginT = []
        for k, (k0, ksz) in enumerate(K1):
            t = wp.tile([ksz, R], F32R, tag=f"ginT{k}", name=f"ginT{k}")
            nc.scalar.dma_start(t[:], ginT_d[k0 : k0 + ksz, :])
            ginT.append(t)

        # ---- u = pw1e.T @ ejkT  (pair part of elem layer 1) ------------
        uT = [wp.tile([128, R], F32, tag=f"uT{m}", name=f"uT{m}") for m in range(2)]
        for m in range(2):
            for rc in range(2):
                psu = psS.tile([128, 512], F32, tag="small", name="psu")
                nc.tensor.matmul(
                    psu[:],
                    pw1e[m],
                    ejkT[:, rc * 512 : (rc + 1) * 512],
                    start=True,
                    stop=True,
                )
                nc.vector.tensor_copy(
                    uT[m][:, rc * 512 : (rc + 1) * 512], psu[:]
                )

        # first-iteration x1, emitted before everything else on ScalarE
        x1_first = []
        for m in range(2):
            o = ap.tile([128, R], F32R, tag=f"x1_{m}", name=f"x1_{m}")
            nc.scalar.activation(o[:], uT[m][:], AF.Silu, bias=vb1[m][:, 0:1])
            x1_first.append(o)

        # ---- geometry MLP (emission staged into the ne loop below) -----
        gm_ctx = {}

        def mlp_half(rhs_tiles, weights, bias_col, out_name):
            ps = psB.tile([128, R], F32, tag="big", name="ps_mlp")
            nk = len(rhs_tiles)
            for rc in range(2):
                for k in range(nk):
                    nc.tensor.matmul(
                        ps[:, rc * 512 : (rc + 1) * 512],
                        weights[k],
                        rhs_tiles[k][:, rc * 512 : (rc + 1) * 512],
                        start=(k == 0),
                        stop=(k == nk - 1),
                    )
            o = wp.tile([128, R], F32R, tag=out_name, name=out_name)
            nc.scalar.activation(
                o[:], ps[:], AF.Silu, bias=bia[:, bias_col : bias_col + 1]
            )
            return o

        def gm_stage_0():
            gm_ctx["x1g0"] = mlp_half(ginT, [w[0] for w in gmW1], GM_B1, "gx1_0")

        def gm_stage_1():
            gm_ctx["x1g1"] = mlp_half(ginT, [w[1] for w in gmW1], GM_B1 + 1, "gx1_1")

        def gm_stage_2():
            x1g = [gm_ctx["x1g0"], gm_ctx["x1g1"]]
            gm_ctx["x2g0"] = mlp_half(x1g, [w[0] for w in gmW2], GM_B2, "gx2_0")

        def gm_stage_3():
            x1g = [gm_ctx["x1g0"], gm_ctx["x1g1"]]
            gm_ctx["x2g1"] = mlp_half(x1g, [w[1] for w in gmW2], GM_B2 + 1, "gx2_1")

        def gm_stage_4():
            x2g = [gm_ctx["x2g0"], gm_ctx["x2g1"]]
            psG = psS.tile([64, R], F32, tag="small", name="psG")
            for rc in range(2):
                for k in range(2):
                    nc.tensor.matmul(
                        psG[:, rc * 512 : (rc + 1) * 512],
                        gmW3[k],
                        x2g[k][:, rc * 512 : (rc + 1) * 512],
                        start=(k == 0),
                        stop=(k == 1),
                    )
            Gtmp = sp.tile([64, R], F32, tag="Gtmp", name="Gtmp")
            nc.vector.tensor_scalar_add(Gtmp[:], psG[:], bia[0:64, GM_B3 : GM_B3 + 1])
            Gm = wp.tile([64, R], F32, tag="Gm", name="Gm")
            nc.vector.tensor_mul(Gm[:], Gtmp[:], pmask2)
            gm_ctx["Gm"] = Gm

        gm_stages = [gm_stage_0, gm_stage_1, gm_stage_2, gm_stage_3, gm_stage_4]

        # ---- final output MLP, one 128-column half at a time -----------
        aggT = wp.tile([64, NE * BPC], F32, tag="aggT")
        out_sb = [sp.tile([128, 256], F32, tag=f"oT_{m}", name=f"oT_{m}") for m in range(2)]

        def final_half(hc):
            cs = slice(hc * 128, (hc + 1) * 128)
            aggR = sp.tile([64, 128], F32R, tag="aggR", name="aggR", bufs=2)
            nc.vector.tensor_copy(aggR[:], aggT[:, cs])
            f1 = []
            for m in range(2):
                ps = psS.tile([128, 128], F32, tag="small", name="psf1")
                nc.tensor.matmul(ps[:], opW1[m], aggR[:], start=True, stop=True)
                o = sp.tile([128, 128], F32R, tag=f"f1_{m}", name=f"f1_{m}")
                nc.scalar.activation(
                    o[:], ps[:], AF.Silu, bias=bia[:, OP_B1 + m : OP_B1 + m + 1]
                )
                f1.append(o)
            for m in range(2):
                ps = psS.tile([128, 128], F32, tag="small", name="psf2")
                for k in range(2):
                    nc.tensor.matmul(
                        ps[:], opW2[k][m], f1[k][:], start=(k == 0), stop=(k == 1)
                    )
                nc.vector.tensor_scalar_add(
                    out_sb[m][:, cs], ps[:], bia[:, OP_B2 + m : OP_B2 + m + 1]
                )
                nc.sync.dma_start(out_d[m, :, cs], out_sb[m][:, cs])

        # ---- the ne loop (x1 prefetched one iteration ahead) -----------
        def emit_x1(ne):
            x1 = []
            for m in range(2):
                o = ap.tile([128, R], F32R, tag=f"x1_{m}", name=f"x1_{m}")
                nc.scalar.activation(
                    o[:], uT[m][:], AF.Silu, bias=vb1[m][:, ne : ne + 1]
                )
                x1.append(o)
            return x1

        def emit_amr(ne, g3sb):
            Gm = gm_ctx["Gm"]
            for b in range(BPC):
                scr = sp.tile([64, PMAX], F32, tag="scr", name="scr")
                nc.vector.affine_mul_reduce(
                    out=scr[:],
                    accum_out=aggT[:, ne * BPC + b : ne * BPC + b + 1],
                    in0=g3sb[:, b * PMAX : (b + 1) * PMAX],
                    in1=Gm[:, b * PMAX : (b + 1) * PMAX],
                    scale=1.0,
                    bias=bia[0:64, PE_B3 : PE_B3 + 1],
                )

        pending = []
        x1 = x1_first
        for ne in range(NE):
            x1_next = emit_x1(ne + 1) if ne + 1 < NE else None
            x2 = []
            for m in range(2):
                ps = psB.tile([128, R], F32, tag="big", name="ps2")
                for rc in range(2):
                    for k in range(2):
                        nc.tensor.matmul(
                            ps[:, rc * 512 : (rc + 1) * 512],
                            peW2[k][m],
                            x1[k][:, rc * 512 : (rc + 1) * 512],
                            start=(k == 0),
                            stop=(k == 1),
                        )
                o = ap.tile([128, R], F32R, tag=f"x2_{m}", name=f"x2_{m}")
                nc.scalar.activation(
                    o[:], ps[:], AF.Silu, bias=bia[:, PE_B2 + m : PE_B2 + m + 1]
                )
                x2.append(o)
            ps3 = psS.tile([64, R], F32, tag="small", name="ps3")
            for rc in range(2):
                for k in range(2):
                    nc.tensor.matmul(
                        ps3[:, rc * 512 : (rc + 1) * 512],
                        peW3[k],
                        x2[k][:, rc * 512 : (rc + 1) * 512],
                        start=(k == 0),
                        stop=(k == 1),
                    )
            g3sb = ap.tile([64, R], F16, tag="g3sb", name="g3sb", bufs=24)
            nc.vector.tensor_copy(g3sb[:], ps3[:])
            if "Gm" in gm_ctx:
                for pne, pg in pending[:4]:
                    emit_amr(pne, pg)
                del pending[:4]
                emit_amr(ne, g3sb)
            else:
                pending.append((ne, g3sb))
            if ne >= 6 and (ne - 6) % 2 == 0 and (ne - 6) // 2 < len(gm_stages):
                gm_stages[(ne - 6) // 2]()
            x1 = x1_next
            if ne == 33:
                final_half(0)
        final_half(1)

    nc.compile()
    return nc


def _get_nc():
    if "nc" not in _CACHE:
        _CACHE["nc"] = _build()
    return _CACHE["nc"]


# --------------------------------------------------------------------------
# entry point
# --------------------------------------------------------------------------

def kernel(
    h,
    z,
    pos,
    mask,
    e_feat,
    z_emb,
    gm_W1,
    gm_b1,
    gm_W2,
    gm_b2,
    gm_W3,
    gm_b3,
    pe_W1,
    pe_b1,
    pe_W2,
    pe_b2,
    pe_W3,
    pe_b3,
    op_W1,
    op_b1,
    op_W2,
    op_b2,
    absorber_index=0,
):
    h = np.asarray(h, np.float32)
    z = np.asarray(z).astype(np.int64)
    pos = np.asarray(pos, np.float32)
    mask = np.asarray(mask).astype(bool)
    e_feat = np.asarray(e_feat, np.float32)
    z_emb = np.asarray(z_emb, np.float32)
    gm_W1 = np.asarray(gm_W1, np.float32)
    gm_b1 = np.asarray(gm_b1, np.float32)
    gm_W2 = np.asarray(gm_W2, np.float32)
    gm_b2 = np.asarray(gm_b2, np.float32)
    gm_W3 = np.asarray(gm_W3, np.float32)
    gm_b3 = np.asarray(gm_b3, np.float32)
    pe_W1 = np.asarray(pe_W1, np.float32)
    pe_b1 = np.asarray(pe_b1, np.float32)
    pe_W2 = np.asarray(pe_W2, np.float32)
    pe_b2 = np.asarray(pe_b2, np.float32)
    pe_W3 = np.asarray(pe_W3, np.float32)
    pe_b3 = np.asarray(pe_b3, np.float32)
    op_W1 = np.asarray(op_W1, np.float32)
    op_b1 = np.asarray(op_b1, np.float32)
    op_W2 = np.asarray(op_W2, np.float32)
    op_b2 = np.asarray(op_b2, np.float32)

    geom_in, ejk, pmask = _host_prep(h, z, pos, mask, z_emb, absorber_index)

    # v[ne] = e_feat @ pe_W1[64:] + pe_b1, the ne-dependent layer-1 bias
    vb1_full = (e_feat @ pe_W1[2 * ZEMB :] + pe_b1).astype(np.float32)  # [NE, PH]
    vb1 = vb1_full.T.reshape(2, 128, NE)  # [mh, 128, NE]

    biases = np.zeros((128, NBIAS), np.float32)
    biases[:, GM_B1] = gm_b1[:128]
    biases[:, GM_B1 + 1] = gm_b1[128:]
    biases[:, GM_B2] = gm_b2[:128]
    biases[:, GM_B2 + 1] = gm_b2[128:]
    biases[0:64, GM_B3] = gm_b3
    biases[:, PE_B2] = pe_b2[:128]
    biases[:, PE_B2 + 1] = pe_b2[128:]
    biases[0:64, PE_B3] = pe_b3
    biases[:, OP_B1] = op_b1[:128]
    biases[:, OP_B1 + 1] = op_b1[128:]
    biases[:, OP_B2] = op_b2[:128]
    biases[:, OP_B2 + 1] = op_b2[128:]

    wpack = np.zeros((128, WCOLS), np.float32)

    def put(c0, w, msplit=True):  # w: [K, M], tiles of [<=128, 128-cols]
        kk, mm = w.shape
        for k in range(0, kk, 128):
            ksz = min(128, kk - k)
            for m in range(0, mm, 128):
                msz = min(128, mm - m)
                col = c0 + (k // 128) * mm + m
                wpack[0:ksz, col : col + msz] = w[k : k + ksz, m : m + msz]

    put(W_PEW2, pe_W2)
    put(W_PEW3, pe_W3)
    put(W_GMW1, gm_W1)
    put(W_GMW2, gm_W2)
    put(W_GMW3, gm_W3)
    put(W_OPW1, op_W1)
    put(W_OPW2, op_W2)

    in_maps = []
    for c in range(N_CORES):
        sl = slice(c * BPC, (c + 1) * BPC)
        gi = geom_in[sl]  # [BPC, P, 353]
        ginT = np.ascontiguousarray(gi.reshape(R, GIN).T)  # [353, R]
        dpack = np.zeros((128, DCOLS), np.float32)
        dpack[0:64, 0:R] = ejk[sl].reshape(R, 2 * ZEMB).T
        pm2 = np.broadcast_to(
            pmask[sl].reshape(1, R).astype(np.float32), (64, R)
        )
        dpack[0:64, D_PW1 : D_PW1 + PH] = pe_W1[: 2 * ZEMB]
        dpack[:, D_VB1 : D_VB1 + NE] = vb1[0]
        dpack[:, D_VB1 + NE : D_VB1 + 2 * NE] = vb1[1]
        dpack[:, D_BIA : D_BIA + NBIAS] = biases
        in_maps.append({"ginT": ginT, "dpack": dpack, "wpack": wpack, "pmask": np.ascontiguousarray(pm2)})

    nc = _get_nc()
    res = run_bass_kernel_spmd(nc, in_maps, list(range(N_CORES)), trace=TRACE)
    _CACHE["last_result"] = res

    out = np.empty((B, NE, OUT), np.float32)
    for c in range(N_CORES):
        oc = res.results[c]["out"]  # [2, 128, 256] = (mh, o, ne*BPC+b)
        oc = oc.reshape(OUT, NE, BPC)  # [256, 64, 4]
        out[c * BPC : (c + 1) * BPC] = oc.transpose(2, 1, 0)
    return out


# revision 18
# speedup vs baseline: 1.0872x; 1.0284x over previous
"""Trainium2 Bass kernel for nn_AbsorberPathAggregator.

Contract: kernel(**inputs) takes the FULL unsharded inputs (as produced by
setup_inputs()) and returns the FULL [B, NE, OUT] float32 output.

Strategy (pure data parallel over B across 8 NeuronCores, 4 samples/core):
  - Host (numpy): pair enumeration (argsort of valid triu pairs), gathers,
    RBF features -> geom_inT [353, 1024] per core; ejkT [64, 1024]; pmask.
    This is cheap index/bookkeeping work; all FLOP-heavy MLPs run on device.
  - Device per core, activations transposed [feature, row], rows = (b, p):
      G = gm-MLP(geom_inT) masked by pmask        (geometry path)
      u = pe_W1[:64].T @ ejkT                     (pair part of elem layer 1)
      for ne in 0..63:  x1 = Silu(u + v[ne]); x2 = Silu(W2.T x1)
                        g3 = W3.T x2;  agg[:, ne] += sum_p (g3+b3)*G
      out = Silu(opW1.T agg + b1) -> opW2 + b2 -> DMA out
    g3/G/agg live in a [128=(rc,s), 512=(b2,p)] layout (rc = sample-pair,
    placed via PE column tiling) so the fused affine_mul_reduce epilogue is
    2 ops of [128, 256] per ne. Matmuls use float32r (full-rate fp32-reduced);
    Silu runs on ScalarE with the free per-partition bias operand (ScalarE is
    the bottleneck engine at ~273 us busy).
"""

import sys

for _p in (
    "/root/.axon_site",
    "/root/.axon_site/_ro/trn_rl_repo",
    "/root/.axon_site/_ro/pypackages",
    "/opt/trn_rl_repo",
):
    if _p not in sys.path:
        sys.path.append(_p)

import numpy as np

from contextlib import ExitStack

import concourse.tile as tile
from concourse import bacc, mybir
from concourse.bass_utils import run_bass_kernel_spmd

F32 = mybir.dt.float32
F32R = mybir.dt.float32r
F16 = mybir.dt.float16
AF = mybir.ActivationFunctionType
ALU = mybir.AluOpType

B, N, H = 32, 64, 128
RBF_DIM, GH, SCATTER, OUT = 32, 256, 64, 256
CUTOFF, PMAX = 6.0, 256
ZEMB, EDIM, NE, PH = 32, 32, 64, 256
GIN = 2 * H + 3 * RBF_DIM + 1  # 353

N_CORES = 8
BPC = B // N_CORES  # 4 samples per core
R = BPC * PMAX  # 1024 rows per core

TRACE = False  # set by test harness for NTFF profiling
_CACHE = {}


# --------------------------------------------------------------------------
# host-side preprocessing (numpy, mirrors the reference's indexing exactly)
# --------------------------------------------------------------------------

def _rbf(x):
    centers = np.linspace(0.0, CUTOFF, RBF_DIM, dtype=np.float32)
    delta = CUTOFF / (RBF_DIM - 1)
    gamma = 1.0 / (delta * delta + 1e-12)
    d = x[..., None] - centers
    return np.exp((-gamma) * d * d).astype(np.float32)


def _norm(v):
    return np.sqrt((v * v).sum(-1) + np.float32(1e-12))


def _host_prep(h, z, pos, mask, z_emb, absorber_index):
    ai = int(absorber_index)
    pos0 = pos[:, ai][:, None, :]  # [B,1,3]
    r = _norm(pos - pos0)  # [B,N]
    valid = mask & (np.arange(N)[None, :] != ai) & (r <= np.float32(CUTOFF))
    ju, ku = np.triu_indices(N, k=1)
    pv = valid[:, ju] & valid[:, ku]  # [B, Np]
    order = np.argsort(~pv, axis=1, kind="stable")[:, :PMAX]
    pmask = np.take_along_axis(pv, order, axis=1)  # [B,PMAX]
    j_idx = np.where(pmask, ju[order], 0)
    k_idx = np.where(pmask, ku[order], 0)
    bidx = np.arange(B)[:, None]
    hj, hk = h[bidx, j_idx], h[bidx, k_idx]  # [B,P,H]
    posj, posk = pos[bidx, j_idx], pos[bidx, k_idx]  # [B,P,3]
    vj, vk, vjk = posj - pos0, posk - pos0, posk - posj
    r0j, r0k, rjk = _norm(vj), _norm(vk), _norm(vjk)
    uj = vj / np.maximum(r0j[..., None], np.float32(1e-8))
    uk = vk / np.maximum(r0k[..., None], np.float32(1e-8))
    cosang = np.clip((uj * uk).sum(-1, keepdims=True), -1.0, 1.0).astype(np.float32)
    geom_in = np.concatenate(
        [
            hj,
            hk,
            _rbf(np.minimum(r0j, np.float32(CUTOFF))),
            _rbf(np.minimum(r0k, np.float32(CUTOFF))),
            _rbf(np.minimum(rjk, np.float32(CUTOFF))),
            cosang,
        ],
        axis=-1,
    ).astype(np.float32)  # [B, P, 353]
    ejk = np.concatenate([z_emb[z[bidx, j_idx]], z_emb[z[bidx, k_idx]]], axis=-1)
    return geom_in, ejk.astype(np.float32), pmask


# --------------------------------------------------------------------------
# device kernel
# --------------------------------------------------------------------------

# bias-pack column indices (within the dpack bias block)
GM_B1, GM_B2, GM_B3, PE_B2, PE_B3, OP_B1, OP_B2 = 0, 2, 4, 5, 7, 8, 10
NBIAS = 12

# dpack [128, DCOLS]: rows 0:64 cols 0:R = ejkT; rows 0:64 cols R:R+256 =
# pe_W1[:64]; cols R+256:R+384 = vb1 (2 mh halves of 64); last NBIAS = biases.
D_PW1 = R
D_VB1 = R + 256
D_BIA = R + 384
DCOLS = R + 384 + NBIAS

# wpack [128, WCOLS] column layout: peW2(512) peW3rc(512) gmW1(768)
# gmW2(512) gmW3rc(512) opW1(256, duplicated on rows 64:128) opW2(512).
# W3 tiles are stored zero-padded to [128,128] as [W3|0] (rc0) and [0|W3]
# (rc1) so the scatter layer lands both rc halves in one [128,512] psum
# without PE tile_position.
W_PEW2 = 0
W_PEW3 = 512
W_GMW1 = 1024
W_GMW2 = 1792
W_GMW3 = 2304
W_OPW1 = 2816
W_OPW1H = 3072
W_OPW2 = 3328
WCOLS = 3840
W_SPLIT = 1024  # pe weights first; loaded ahead of the rest


def _build():
    nc = bacc.Bacc("TRN2", target_bir_lowering=False, debug=False, num_devices=N_CORES)

    ginT_d = nc.dram_tensor("ginT", [GIN, R], F32R, kind="ExternalInput").ap()
    pmask_d = nc.dram_tensor("pmask", [128, 512], F32, kind="ExternalInput").ap()
    dpack_d = nc.dram_tensor("dpack", [128, DCOLS], F32R, kind="ExternalInput").ap()
    wpack_d = nc.dram_tensor("wpack", [128, WCOLS], F32R, kind="ExternalInput").ap()
    out_d = nc.dram_tensor("out", [2, 128, 256], F32, kind="ExternalOutput").ap()

    K1 = [(0, 128), (128, 128), (256, GIN - 256)]  # gm layer-1 k tiles

    with tile.TileContext(nc) as tc, ExitStack() as ctx:
        wp = ctx.enter_context(tc.tile_pool(name="wp", bufs=1))
        ap = ctx.enter_context(tc.tile_pool(name="ap", bufs=3))
        sp = ctx.enter_context(tc.tile_pool(name="sp", bufs=2))
        psB = ctx.enter_context(tc.tile_pool(name="psB", bufs=3, space="PSUM"))
        psS = ctx.enter_context(tc.tile_pool(name="psS", bufs=2, space="PSUM"))

        # ---- loads (sync HWDGE; dpack is on the critical path) ----------
        dpack = wp.tile([128, DCOLS], F32R, tag="dpack")
        nc.sync.dma_start(dpack[0:64, 0:R], dpack_d[0:64, 0:R])
        nc.sync.dma_start(dpack[:, R:DCOLS], dpack_d[:, R:DCOLS])
        wpack = wp.tile([128, WCOLS], F32R, tag="wpack")
        nc.sync.dma_start(wpack[:, 0:W_SPLIT], wpack_d[:, 0:W_SPLIT])
        nc.sync.dma_start(wpack[:, W_SPLIT:W_GMW2], wpack_d[:, W_SPLIT:W_GMW2])
        nc.sync.dma_start(wpack[:, W_GMW2:], wpack_d[:, W_GMW2:])
        pmask2t = wp.tile([128, 512], F32, tag="pmask2t")
        nc.sync.dma_start(pmask2t[:], pmask_d)
        pmask2 = pmask2t[:]

        ejkT = dpack[0:64, 0:R]
        pw1e = [dpack[0:64, D_PW1 + m * 128 : D_PW1 + (m + 1) * 128] for m in range(2)]
        vb1 = [
            dpack[:, D_VB1 + m * NE : D_VB1 + (m + 1) * NE].bitcast(F32)
            for m in range(2)
        ]
        bia = dpack[:, D_BIA : D_BIA + NBIAS].bitcast(F32)

        def wtile(c0, ksz, m):
            return wpack[0:ksz, c0 + m * 128 : c0 + (m + 1) * 128]

        peW2 = [[wtile(W_PEW2 + k * 256, 128, m) for m in range(2)] for k in range(2)]
        peW3 = [
            [
                wpack[0:128, W_PEW3 + k * 256 + rc * 128 : W_PEW3 + k * 256 + (rc + 1) * 128]
                for rc in range(2)
            ]
            for k in range(2)
        ]
        gmW1 = [
            [wtile(W_GMW1 + k * 256, K1[k][1], m) for m in range(2)] for k in range(3)
        ]
        gmW2 = [[wtile(W_GMW2 + k * 256, 128, m) for m in range(2)] for k in range(2)]
        gmW3 = [
            [
                wpack[0:128, W_GMW3 + k * 256 + rc * 128 : W_GMW3 + k * 256 + (rc + 1) * 128]
                for rc in range(2)
            ]
            for k in range(2)
        ]
        opW1 = [
            wpack[0:128, W_OPW1 + m * 128 : W_OPW1 + (m + 1) * 128] for m in range(2)
        ]
        opW1h = [
            wpack[0:128, W_OPW1H + m * 128 : W_OPW1H + (m + 1) * 128] for m in range(2)
        ]
        opW2 = [[wtile(W_OPW2 + k * 256, 128, m) for m in range(2)] for k in range(2)]

        ginT = []
        for k, (k0, ksz) in enumerate(K1):
            t = wp.tile([ksz, R], F32R, tag=f"ginT{k}", name=f"ginT{k}")
            nc.scalar.dma_start(t[:], ginT_d[k0 : k0 + ksz, :])
            ginT.append(t)

        # ---- u = pw1e.T @ ejkT  (pair part of elem layer 1) ------------
        uT = [wp.tile([128, R], F32, tag=f"uT{m}", name=f"uT{m}") for m in range(2)]
        for m in range(2):
            for rc in range(2):
                psu = psS.tile([128, 512], F32, tag="small", name="psu")
                nc.tensor.matmul(
                    psu[:],
                    pw1e[m],
                    ejkT[:, rc * 512 : (rc + 1) * 512],
                    start=True,
                    stop=True,
                )
                nc.vector.tensor_copy(uT[m][:, rc * 512 : (rc + 1) * 512], psu[:])

        # first-iteration x1, emitted before everything else on ScalarE
        x1_first = []
        for m in range(2):
            o = ap.tile([128, R], F32R, tag=f"x1_{m}", name=f"x1_{m}")
            nc.scalar.activation(o[:], uT[m][:], AF.Silu, bias=vb1[m][:, 0:1])
            x1_first.append(o)

        # scatter-layer matmul: write [64 x 512] results of both rc halves
        # into one [128=(rc,s), 512=(b2,p)] psum tile via PE column tiling
        def scatter_mm(ps, weights, rhs_tiles):
            for rc in range(2):
                for k in range(2):
                    nc.tensor.matmul(
                        ps[:],
                        weights[k][rc],
                        rhs_tiles[k][:, rc * 512 : (rc + 1) * 512],
                        start=(rc == 0 and k == 0),
                        stop=(rc == 1 and k == 1),
                    )

        # ---- geometry MLP (emission staged into the ne loop below) -----
        gm_ctx = {}

        def mlp_half(rhs_tiles, weights, bias_col, out_name):
            ps = psB.tile([128, R], F32, tag="big", name="ps_mlp")
            nk = len(rhs_tiles)
            for rc in range(2):
                for k in range(nk):
                    nc.tensor.matmul(
                        ps[:, rc * 512 : (rc + 1) * 512],
                        weights[k],
                        rhs_tiles[k][:, rc * 512 : (rc + 1) * 512],
                        start=(k == 0),
                        stop=(k == nk - 1),
                    )
            o = wp.tile([128, R], F32R, tag=out_name, name=out_name)
            nc.scalar.activation(
                o[:], ps[:], AF.Silu, bias=bia[:, bias_col : bias_col + 1]
            )
            return o

        def gm_stage_0():
            gm_ctx["x1g0"] = mlp_half(ginT, [w[0] for w in gmW1], GM_B1, "gx1_0")

        def gm_stage_1():
            gm_ctx["x1g1"] = mlp_half(ginT, [w[1] for w in gmW1], GM_B1 + 1, "gx1_1")

        def gm_stage_2():
            x1g = [gm_ctx["x1g0"], gm_ctx["x1g1"]]
            gm_ctx["x2g0"] = mlp_half(x1g, [w[0] for w in gmW2], GM_B2, "gx2_0")

        def gm_stage_3():
            x1g = [gm_ctx["x1g0"], gm_ctx["x1g1"]]
            gm_ctx["x2g1"] = mlp_half(x1g, [w[1] for w in gmW2], GM_B2 + 1, "gx2_1")

        def gm_stage_4():
            x2g = [gm_ctx["x2g0"], gm_ctx["x2g1"]]
            psG = psS.tile([128, 512], F32, tag="small", name="psG")
            scatter_mm(psG, gmW3, x2g)
            Gtmp = sp.tile([128, 512], F32, tag="Gtmp", name="Gtmp")
            nc.vector.tensor_scalar_add(Gtmp[:], psG[:], bia[:, GM_B3 : GM_B3 + 1])
            Gm = wp.tile([128, 512], F32, tag="Gm", name="Gm")
            nc.vector.tensor_mul(Gm[:], Gtmp[:], pmask2)
            gm_ctx["Gm"] = Gm

        gm_stages = [gm_stage_0, gm_stage_1, gm_stage_2, gm_stage_3, gm_stage_4]

        # ---- final output MLP, one 64-column (of 128) chunk at a time --
        # aggT [128=(rc,s), 128=(ne*2+b2)]
        aggT = wp.tile([128, NE * 2], F32, tag="aggT")
        out_sb = [sp.tile([128, 256], F32, tag=f"oT_{m}", name=f"oT_{m}") for m in range(2)]

        def final_half(hc):
            cs = slice(hc * 64, (hc + 1) * 64)
            aggR = sp.tile([128, 64], F32R, tag="aggR", name="aggR", bufs=2)
            nc.vector.tensor_copy(aggR[:], aggT[:, cs])
            f1 = []
            for m in range(2):
                ps = psS.tile([128, 128], F32, tag="small", name="psf1")
                nc.tensor.matmul(ps[:, 0:64], opW1[m], aggR[:], start=True, stop=True)
                nc.tensor.matmul(
                    ps[:, 64:128], opW1h[m], aggR[:], start=True, stop=True
                )
                o = sp.tile([128, 128], F32R, tag=f"f1_{m}", name=f"f1_{m}")
                nc.scalar.activation(
                    o[:], ps[:], AF.Silu, bias=bia[:, OP_B1 + m : OP_B1 + m + 1]
                )
                f1.append(o)
            for m in range(2):
                ps = psS.tile([128, 128], F32, tag="small", name="psf2")
                for k in range(2):
                    nc.tensor.matmul(
                        ps[:], opW2[k][m], f1[k][:], start=(k == 0), stop=(k == 1)
                    )
                for rc in range(2):
                    nc.vector.tensor_scalar_add(
                        out_sb[m][:, rc * 128 + hc * 64 : rc * 128 + (hc + 1) * 64],
                        ps[:, rc * 64 : (rc + 1) * 64],
                        bia[:, OP_B2 + m : OP_B2 + m + 1],
                    )
                nc.sync.dma_start(
                    out_d[m, :, hc * 64 : (hc + 1) * 64],
                    out_sb[m][:, hc * 64 : (hc + 1) * 64],
                )
                nc.sync.dma_start(
                    out_d[m, :, 128 + hc * 64 : 128 + (hc + 1) * 64],
                    out_sb[m][:, 128 + hc * 64 : 128 + (hc + 1) * 64],
                )

        # ---- the ne loop (x1 prefetched one iteration ahead) -----------
        def emit_x1(ne):
            x1 = []
            for m in range(2):
                o = ap.tile([128, R], F32R, tag=f"x1_{m}", name=f"x1_{m}")
                nc.scalar.activation(
                    o[:], uT[m][:], AF.Silu, bias=vb1[m][:, ne : ne + 1]
                )
                x1.append(o)
            return x1

        def emit_amr(ne, g3sb):
            Gm = gm_ctx["Gm"]
            for b2 in range(2):
                scr = sp.tile([128, PMAX], F32, tag="scr", name="scr")
                nc.vector.affine_mul_reduce(
                    out=scr[:],
                    accum_out=aggT[:, ne * 2 + b2 : ne * 2 + b2 + 1],
                    in0=g3sb[:, b2 * PMAX : (b2 + 1) * PMAX],
                    in1=Gm[:, b2 * PMAX : (b2 + 1) * PMAX],
                    scale=1.0,
                    bias=bia[:, PE_B3 : PE_B3 + 1],
                )

        pending = []
        x1 = x1_first
        for ne in range(NE):
            x1_next = emit_x1(ne + 1) if ne + 1 < NE else None
            x2 = []
            for m in range(2):
                ps = psB.tile([128, R], F32, tag="big", name="ps2")
                for rc in range(2):
                    for k in range(2):
                        nc.tensor.matmul(
                            ps[:, rc * 512 : (rc + 1) * 512],
                            peW2[k][m],
                            x1[k][:, rc * 512 : (rc + 1) * 512],
                            start=(k == 0),
                            stop=(k == 1),
                        )
                o = ap.tile([128, R], F32R, tag=f"x2_{m}", name=f"x2_{m}")
                nc.scalar.activation(
                    o[:], ps[:], AF.Silu, bias=bia[:, PE_B2 + m : PE_B2 + m + 1]
                )
                x2.append(o)
            ps3 = psS.tile([128, 512], F32, tag="small", name="ps3")
            scatter_mm(ps3, peW3, x2)
            g3sb = ap.tile([128, 512], F16, tag="g3sb", name="g3sb", bufs=20)
            nc.vector.tensor_copy(g3sb[:], ps3[:])
            if "Gm" in gm_ctx:
                for pne, pg in pending[:4]:
                    emit_amr(pne, pg)
                del pending[:4]
                emit_amr(ne, g3sb)
            else:
                pending.append((ne, g3sb))
            if ne >= 6 and (ne - 6) % 2 == 0 and (ne - 6) // 2 < len(gm_stages):
                gm_stages[(ne - 6) // 2]()
            x1 = x1_next
            if ne == 33:
                final_half(0)
        final_half(1)

    nc.compile()
    return nc


def _get_nc():
    if "nc" not in _CACHE:
        _CACHE["nc"] = _build()
    return _CACHE["nc"]


# --------------------------------------------------------------------------
# entry point
# --------------------------------------------------------------------------

def kernel(
    h,
    z,
    pos,
    mask,
    e_feat,
    z_emb,
    gm_W1,
    gm_b1,
    gm_W2,
    gm_b2,
    gm_W3,
    gm_b3,
    pe_W1,
    pe_b1,
    pe_W2,
    pe_b2,
    pe_W3,
    pe_b3,
    op_W1,
    op_b1,
    op_W2,
    op_b2,
    absorber_index=0,
):
    h = np.asarray(h, np.float32)
    z = np.asarray(z).astype(np.int64)
    pos = np.asarray(pos, np.float32)
    mask = np.asarray(mask).astype(bool)
    e_feat = np.asarray(e_feat, np.float32)
    z_emb = np.asarray(z_emb, np.float32)
    gm_W1 = np.asarray(gm_W1, np.float32)
    gm_b1 = np.asarray(gm_b1, np.float32)
    gm_W2 = np.asarray(gm_W2, np.float32)
    gm_b2 = np.asarray(gm_b2, np.float32)
    gm_W3 = np.asarray(gm_W3, np.float32)
    gm_b3 = np.asarray(gm_b3, np.float32)
    pe_W1 = np.asarray(pe_W1, np.float32)
    pe_b1 = np.asarray(pe_b1, np.float32)
    pe_W2 = np.asarray(pe_W2, np.float32)
    pe_b2 = np.asarray(pe_b2, np.float32)
    pe_W3 = np.asarray(pe_W3, np.float32)
    pe_b3 = np.asarray(pe_b3, np.float32)
    op_W1 = np.asarray(op_W1, np.float32)
    op_b1 = np.asarray(op_b1, np.float32)
    op_W2 = np.asarray(op_W2, np.float32)
    op_b2 = np.asarray(op_b2, np.float32)

    geom_in, ejk, pmask = _host_prep(h, z, pos, mask, z_emb, absorber_index)

    # v[ne] = e_feat @ pe_W1[64:] + pe_b1, the ne-dependent layer-1 bias
    vb1_full = (e_feat @ pe_W1[2 * ZEMB :] + pe_b1).astype(np.float32)  # [NE, PH]
    vb1 = vb1_full.T.reshape(2, 128, NE)

    biases = np.zeros((128, NBIAS), np.float32)
    biases[:, GM_B1] = gm_b1[:128]
    biases[:, GM_B1 + 1] = gm_b1[128:]
    biases[:, GM_B2] = gm_b2[:128]
    biases[:, GM_B2 + 1] = gm_b2[128:]
    biases[0:64, GM_B3] = gm_b3
    biases[64:128, GM_B3] = gm_b3
    biases[:, PE_B2] = pe_b2[:128]
    biases[:, PE_B2 + 1] = pe_b2[128:]
    biases[0:64, PE_B3] = pe_b3
    biases[64:128, PE_B3] = pe_b3
    biases[:, OP_B1] = op_b1[:128]
    biases[:, OP_B1 + 1] = op_b1[128:]
    biases[:, OP_B2] = op_b2[:128]
    biases[:, OP_B2 + 1] = op_b2[128:]

    wpack = np.zeros((128, WCOLS), np.float32)

    def put(c0, w):  # w: [K, M], tiles of [<=128 rows, 128 cols]
        kk, mm = w.shape
        for k in range(0, kk, 128):
            ksz = min(128, kk - k)
            for m in range(0, mm, 128):
                msz = min(128, mm - m)
                col = c0 + (k // 128) * mm + m
                wpack[0:ksz, col : col + msz] = w[k : k + ksz, m : m + msz]

    def put_rc(c0, w):  # w [256, 64] -> 4 zero-padded [128,128] tiles
        for k in range(2):
            for rc in range(2):
                col = c0 + k * 256 + rc * 128
                wpack[0:128, col + rc * 64 : col + rc * 64 + 64] = w[
                    k * 128 : (k + 1) * 128, :
                ]

    put(W_PEW2, pe_W2)
    put_rc(W_PEW3, pe_W3)
    put_rc(W_GMW3, gm_W3)

    put(W_GMW1, gm_W1)
    put(W_GMW2, gm_W2)

    put(W_OPW1, op_W1)  # rows 0:64, rows 64:128 stay zero
    wpack[64:128, W_OPW1H : W_OPW1H + 256] = wpack[0:64, W_OPW1 : W_OPW1 + 256]
    put(W_OPW2, op_W2)

    in_maps = []
    for c in range(N_CORES):
        sl = slice(c * BPC, (c + 1) * BPC)
        gi = geom_in[sl]  # [BPC, P, 353]
        ginT = np.ascontiguousarray(gi.reshape(R, GIN).T)  # [353, R]
        dpack = np.zeros((128, DCOLS), np.float32)
        dpack[0:64, 0:R] = ejk[sl].reshape(R, 2 * ZEMB).T
        dpack[0:64, D_PW1 : D_PW1 + PH] = pe_W1[: 2 * ZEMB]
        dpack[:, D_VB1 : D_VB1 + NE] = vb1[0]
        dpack[:, D_VB1 + NE : D_VB1 + 2 * NE] = vb1[1]
        dpack[:, D_BIA : D_BIA + NBIAS] = biases
        # pmask in the [128=(rc,s), 512=(b2,p)] layout: row rc*64+s holds
        # sample (2*rc+b2)'s mask in column block b2
        pmc = pmask[sl].astype(np.float32)  # [4, 256]
        pm2 = np.zeros((128, 512), np.float32)
        for rc in range(2):
            for b2 in range(2):
                pm2[rc * 64 : (rc + 1) * 64, b2 * 256 : (b2 + 1) * 256] = pmc[
                    2 * rc + b2
                ][None, :]
        in_maps.append({"ginT": ginT, "dpack": dpack, "wpack": wpack, "pmask": pm2})

    nc = _get_nc()
    res = run_bass_kernel_spmd(nc, in_maps, list(range(N_CORES)), trace=TRACE)
    _CACHE["last_result"] = res

    out = np.empty((B, NE, OUT), np.float32)
    for c in range(N_CORES):
        oc = res.results[c]["out"]  # [2, 128, 256]; col = rc*128 + ne*2 + b2
        oc = oc.reshape(OUT, 2, NE, 2)  # [o, rc, ne, b2]
        out[c * BPC : (c + 1) * BPC] = oc.transpose(1, 3, 2, 0).reshape(BPC, NE, OUT)
    return out


# revision 19
# speedup vs baseline: 1.0941x; 1.0063x over previous
"""Trainium2 Bass kernel for nn_AbsorberPathAggregator.

Contract: kernel(**inputs) takes the FULL unsharded inputs (as produced by
setup_inputs()) and returns the FULL [B, NE, OUT] float32 output.

Strategy (pure data parallel over B across 8 NeuronCores, 4 samples/core):
  - Host (numpy): pair enumeration (argsort of valid triu pairs), gathers,
    RBF features -> geom_inT [353, 1024] per core; ejkT [64, 1024]; pmask.
    This is cheap index/bookkeeping work; all FLOP-heavy MLPs run on device.
  - Device per core, activations transposed [feature, row], rows = (b, p):
      G = gm-MLP(geom_inT) masked by pmask        (geometry path)
      u = pe_W1[:64].T @ ejkT                     (pair part of elem layer 1)
      for ne in 0..63:  x1 = Silu(u + v[ne]); x2 = Silu(W2.T x1)
                        g3 = W3.T x2;  agg[:, ne] += sum_p (g3+b3)*G
      out = Silu(opW1.T agg + b1) -> opW2 + b2 -> DMA out
    g3/G/agg live in a [128=(rc,s), 512=(b2,p)] layout (rc = sample-pair,
    placed via PE column tiling) so the fused affine_mul_reduce epilogue is
    2 ops of [128, 256] per ne. Matmuls use float32r (full-rate fp32-reduced);
    Silu runs on ScalarE with the free per-partition bias operand (ScalarE is
    the bottleneck engine at ~273 us busy).
"""

import sys

for _p in (
    "/root/.axon_site",
    "/root/.axon_site/_ro/trn_rl_repo",
    "/root/.axon_site/_ro/pypackages",
    "/opt/trn_rl_repo",
):
    if _p not in sys.path:
        sys.path.append(_p)

import numpy as np

from contextlib import ExitStack

import concourse.tile as tile
from concourse import bacc, mybir
from concourse.bass_utils import run_bass_kernel_spmd

F32 = mybir.dt.float32
F32R = mybir.dt.float32r
F16 = mybir.dt.float16
AF = mybir.ActivationFunctionType
ALU = mybir.AluOpType

B, N, H = 32, 64, 128
RBF_DIM, GH, SCATTER, OUT = 32, 256, 64, 256
CUTOFF, PMAX = 6.0, 256
ZEMB, EDIM, NE, PH = 32, 32, 64, 256
GIN = 2 * H + 3 * RBF_DIM + 1  # 353

N_CORES = 8
BPC = B // N_CORES  # 4 samples per core
R = BPC * PMAX  # 1024 rows per core

TRACE = False  # set by test harness for NTFF profiling
_CACHE = {}


# --------------------------------------------------------------------------
# host-side preprocessing (numpy, mirrors the reference's indexing exactly)
# --------------------------------------------------------------------------

def _rbf(x):
    centers = np.linspace(0.0, CUTOFF, RBF_DIM, dtype=np.float32)
    delta = CUTOFF / (RBF_DIM - 1)
    gamma = 1.0 / (delta * delta + 1e-12)
    d = x[..., None] - centers
    return np.exp((-gamma) * d * d).astype(np.float32)


def _norm(v):
    return np.sqrt((v * v).sum(-1) + np.float32(1e-12))


def _host_prep(h, z, pos, mask, z_emb, absorber_index):
    ai = int(absorber_index)
    pos0 = pos[:, ai][:, None, :]  # [B,1,3]
    r = _norm(pos - pos0)  # [B,N]
    valid = mask & (np.arange(N)[None, :] != ai) & (r <= np.float32(CUTOFF))
    ju, ku = np.triu_indices(N, k=1)
    pv = valid[:, ju] & valid[:, ku]  # [B, Np]
    order = np.argsort(~pv, axis=1, kind="stable")[:, :PMAX]
    pmask = np.take_along_axis(pv, order, axis=1)  # [B,PMAX]
    j_idx = np.where(pmask, ju[order], 0)
    k_idx = np.where(pmask, ku[order], 0)
    bidx = np.arange(B)[:, None]
    hj, hk = h[bidx, j_idx], h[bidx, k_idx]  # [B,P,H]
    posj, posk = pos[bidx, j_idx], pos[bidx, k_idx]  # [B,P,3]
    vj, vk, vjk = posj - pos0, posk - pos0, posk - posj
    r0j, r0k, rjk = _norm(vj), _norm(vk), _norm(vjk)
    uj = vj / np.maximum(r0j[..., None], np.float32(1e-8))
    uk = vk / np.maximum(r0k[..., None], np.float32(1e-8))
    cosang = np.clip((uj * uk).sum(-1, keepdims=True), -1.0, 1.0).astype(np.float32)
    geom_in = np.concatenate(
        [
            hj,
            hk,
            _rbf(np.minimum(r0j, np.float32(CUTOFF))),
            _rbf(np.minimum(r0k, np.float32(CUTOFF))),
            _rbf(np.minimum(rjk, np.float32(CUTOFF))),
            cosang,
        ],
        axis=-1,
    ).astype(np.float32)  # [B, P, 353]
    ejk = np.concatenate([z_emb[z[bidx, j_idx]], z_emb[z[bidx, k_idx]]], axis=-1)
    return geom_in, ejk.astype(np.float32), pmask


# --------------------------------------------------------------------------
# device kernel
# --------------------------------------------------------------------------

# bias-pack column indices (within the dpack bias block)
GM_B1, GM_B2, GM_B3, PE_B2, PE_B3, OP_B1, OP_B2 = 0, 2, 4, 5, 7, 8, 10
NBIAS = 12

# dpack [128, DCOLS]: rows 0:64 cols 0:256 = pe_W1[:64]; rows 0:64 cols
# 256:256+R = ejkT; then vb1 (2 mh halves of 64); last NBIAS = biases.
D_PW1 = 0
D_EJK = 256
D_VB1 = 256 + R
D_BIA = D_VB1 + 128
DCOLS = D_BIA + NBIAS

# wpack [128, WCOLS] column layout: peW2(512) peW3rc(512) gmW1(768)
# gmW2(512) gmW3rc(512) opW1(256, duplicated on rows 64:128) opW2(512).
# W3 tiles are stored zero-padded to [128,128] as [W3|0] (rc0) and [0|W3]
# (rc1) so the scatter layer lands both rc halves in one [128,512] psum
# without PE tile_position.
W_PEW2 = 0
W_PEW3 = 512
W_GMW1 = 1024
W_GMW2 = 1792
W_GMW3 = 2304
W_OPW1 = 2816
W_OPW1H = 3072
W_OPW2 = 3328
WCOLS = 3840
W_SPLIT = 1024  # pe weights first; loaded ahead of the rest


def _build():
    nc = bacc.Bacc("TRN2", target_bir_lowering=False, debug=False, num_devices=N_CORES)

    ginT_d = nc.dram_tensor("ginT", [GIN, R], F32R, kind="ExternalInput").ap()
    pmask_d = nc.dram_tensor("pmask", [128, 512], F32, kind="ExternalInput").ap()
    dpack_d = nc.dram_tensor("dpack", [128, DCOLS], F32R, kind="ExternalInput").ap()
    wpack_d = nc.dram_tensor("wpack", [128, WCOLS], F32R, kind="ExternalInput").ap()
    out_d = nc.dram_tensor("out", [2, 128, 256], F32, kind="ExternalOutput").ap()

    K1 = [(0, 128), (128, 128), (256, GIN - 256)]  # gm layer-1 k tiles

    with tile.TileContext(nc) as tc, ExitStack() as ctx:
        wp = ctx.enter_context(tc.tile_pool(name="wp", bufs=1))
        ap = ctx.enter_context(tc.tile_pool(name="ap", bufs=3))
        sp = ctx.enter_context(tc.tile_pool(name="sp", bufs=2))
        psB = ctx.enter_context(tc.tile_pool(name="psB", bufs=3, space="PSUM"))
        psS = ctx.enter_context(tc.tile_pool(name="psS", bufs=2, space="PSUM"))

        # ---- loads (sync HWDGE; dpack is on the critical path) ----------
        dpack = wp.tile([128, DCOLS], F32R, tag="dpack")
        nc.sync.dma_start(dpack[0:64, 0 : D_EJK + R], dpack_d[0:64, 0 : D_EJK + R])
        nc.sync.dma_start(dpack[:, D_VB1:DCOLS], dpack_d[:, D_VB1:DCOLS])
        wpack = wp.tile([128, WCOLS], F32R, tag="wpack")
        nc.sync.dma_start(wpack[:, 0:W_SPLIT], wpack_d[:, 0:W_SPLIT])
        nc.sync.dma_start(wpack[:, W_SPLIT:W_GMW2], wpack_d[:, W_SPLIT:W_GMW2])
        nc.sync.dma_start(wpack[:, W_GMW2:], wpack_d[:, W_GMW2:])
        pmask2t = wp.tile([128, 512], F32, tag="pmask2t")
        nc.sync.dma_start(pmask2t[:], pmask_d)
        pmask2 = pmask2t[:]

        ejkT = dpack[0:64, D_EJK : D_EJK + R]
        pw1e = [dpack[0:64, D_PW1 + m * 128 : D_PW1 + (m + 1) * 128] for m in range(2)]
        vb1 = [
            dpack[:, D_VB1 + m * NE : D_VB1 + (m + 1) * NE].bitcast(F32)
            for m in range(2)
        ]
        bia = dpack[:, D_BIA : D_BIA + NBIAS].bitcast(F32)

        def wtile(c0, ksz, m):
            return wpack[0:ksz, c0 + m * 128 : c0 + (m + 1) * 128]

        peW2 = [[wtile(W_PEW2 + k * 256, 128, m) for m in range(2)] for k in range(2)]
        peW3 = [
            [
                wpack[0:128, W_PEW3 + k * 256 + rc * 128 : W_PEW3 + k * 256 + (rc + 1) * 128]
                for rc in range(2)
            ]
            for k in range(2)
        ]
        gmW1 = [
            [wtile(W_GMW1 + k * 256, K1[k][1], m) for m in range(2)] for k in range(3)
        ]
        gmW2 = [[wtile(W_GMW2 + k * 256, 128, m) for m in range(2)] for k in range(2)]
        gmW3 = [
            [
                wpack[0:128, W_GMW3 + k * 256 + rc * 128 : W_GMW3 + k * 256 + (rc + 1) * 128]
                for rc in range(2)
            ]
            for k in range(2)
        ]
        opW1 = [
            wpack[0:128, W_OPW1 + m * 128 : W_OPW1 + (m + 1) * 128] for m in range(2)
        ]
        opW1h = [
            wpack[0:128, W_OPW1H + m * 128 : W_OPW1H + (m + 1) * 128] for m in range(2)
        ]
        opW2 = [[wtile(W_OPW2 + k * 256, 128, m) for m in range(2)] for k in range(2)]

        ginT = []
        for k, (k0, ksz) in enumerate(K1):
            t = wp.tile([ksz, R], F32R, tag=f"ginT{k}", name=f"ginT{k}")
            nc.scalar.dma_start(t[:], ginT_d[k0 : k0 + ksz, :])
            ginT.append(t)

        # ---- u = pw1e.T @ ejkT  (pair part of elem layer 1) ------------
        uT = [wp.tile([128, R], F32, tag=f"uT{m}", name=f"uT{m}") for m in range(2)]
        for m in range(2):
            for rc in range(2):
                psu = psS.tile([128, 512], F32, tag="small", name="psu")
                nc.tensor.matmul(
                    psu[:],
                    pw1e[m],
                    ejkT[:, rc * 512 : (rc + 1) * 512],
                    start=True,
                    stop=True,
                )
                nc.vector.tensor_copy(uT[m][:, rc * 512 : (rc + 1) * 512], psu[:])

        # first-iteration x1, emitted before everything else on ScalarE
        x1_first = []
        for m in range(2):
            o = ap.tile([128, R], F32R, tag=f"x1_{m}", name=f"x1_{m}")
            nc.scalar.activation(o[:], uT[m][:], AF.Silu, bias=vb1[m][:, 0:1])
            x1_first.append(o)

        # scatter-layer matmul: write [64 x 512] results of both rc halves
        # into one [128=(rc,s), 512=(b2,p)] psum tile via PE column tiling
        def scatter_mm(ps, weights, rhs_tiles):
            for rc in range(2):
                for k in range(2):
                    nc.tensor.matmul(
                        ps[:],
                        weights[k][rc],
                        rhs_tiles[k][:, rc * 512 : (rc + 1) * 512],
                        start=(rc == 0 and k == 0),
                        stop=(rc == 1 and k == 1),
                    )

        # ---- geometry MLP (emission staged into the ne loop below) -----
        gm_ctx = {}

        def mlp_half(rhs_tiles, weights, bias_col, out_name):
            ps = psB.tile([128, R], F32, tag="big", name="ps_mlp")
            nk = len(rhs_tiles)
            for rc in range(2):
                for k in range(nk):
                    nc.tensor.matmul(
                        ps[:, rc * 512 : (rc + 1) * 512],
                        weights[k],
                        rhs_tiles[k][:, rc * 512 : (rc + 1) * 512],
                        start=(k == 0),
                        stop=(k == nk - 1),
                    )
            o = wp.tile([128, R], F32R, tag=out_name, name=out_name)
            nc.scalar.activation(
                o[:], ps[:], AF.Silu, bias=bia[:, bias_col : bias_col + 1]
            )
            return o

        def gm_stage_0():
            gm_ctx["x1g0"] = mlp_half(ginT, [w[0] for w in gmW1], GM_B1, "gx1_0")

        def gm_stage_1():
            gm_ctx["x1g1"] = mlp_half(ginT, [w[1] for w in gmW1], GM_B1 + 1, "gx1_1")

        def gm_stage_2():
            x1g = [gm_ctx["x1g0"], gm_ctx["x1g1"]]
            gm_ctx["x2g0"] = mlp_half(x1g, [w[0] for w in gmW2], GM_B2, "gx2_0")

        def gm_stage_3():
            x1g = [gm_ctx["x1g0"], gm_ctx["x1g1"]]
            gm_ctx["x2g1"] = mlp_half(x1g, [w[1] for w in gmW2], GM_B2 + 1, "gx2_1")

        def gm_stage_4():
            x2g = [gm_ctx["x2g0"], gm_ctx["x2g1"]]
            psG = psS.tile([128, 512], F32, tag="small", name="psG")
            scatter_mm(psG, gmW3, x2g)
            Gtmp = sp.tile([128, 512], F32, tag="Gtmp", name="Gtmp")
            nc.vector.tensor_scalar_add(Gtmp[:], psG[:], bia[:, GM_B3 : GM_B3 + 1])
            Gm = wp.tile([128, 512], F32, tag="Gm", name="Gm")
            nc.vector.tensor_mul(Gm[:], Gtmp[:], pmask2)
            gm_ctx["Gm"] = Gm

        gm_stages = [gm_stage_0, gm_stage_1, gm_stage_2, gm_stage_3, gm_stage_4]

        # ---- final output MLP, one 64-column (of 128) chunk at a time --
        # aggT [128=(rc,s), 128=(ne*2+b2)]
        aggT = wp.tile([128, NE * 2], F32, tag="aggT")
        out_sb = [sp.tile([128, 256], F32, tag=f"oT_{m}", name=f"oT_{m}") for m in range(2)]

        def final_half(hc):
            cs = slice(hc * 64, (hc + 1) * 64)
            aggR = sp.tile([128, 64], F32R, tag="aggR", name="aggR", bufs=2)
            nc.vector.tensor_copy(aggR[:], aggT[:, cs])
            f1 = []
            for m in range(2):
                ps = psS.tile([128, 128], F32, tag="small", name="psf1")
                nc.tensor.matmul(ps[:, 0:64], opW1[m], aggR[:], start=True, stop=True)
                nc.tensor.matmul(
                    ps[:, 64:128], opW1h[m], aggR[:], start=True, stop=True
                )
                o = sp.tile([128, 128], F32R, tag=f"f1_{m}", name=f"f1_{m}")
                nc.scalar.activation(
                    o[:], ps[:], AF.Silu, bias=bia[:, OP_B1 + m : OP_B1 + m + 1]
                )
                f1.append(o)
            for m in range(2):
                ps = psS.tile([128, 128], F32, tag="small", name="psf2")
                for k in range(2):
                    nc.tensor.matmul(
                        ps[:], opW2[k][m], f1[k][:], start=(k == 0), stop=(k == 1)
                    )
                for rc in range(2):
                    nc.vector.tensor_scalar_add(
                        out_sb[m][:, rc * 128 + hc * 64 : rc * 128 + (hc + 1) * 64],
                        ps[:, rc * 64 : (rc + 1) * 64],
                        bia[:, OP_B2 + m : OP_B2 + m + 1],
                    )
                nc.sync.dma_start(
                    out_d[m, :, hc * 64 : (hc + 1) * 64],
                    out_sb[m][:, hc * 64 : (hc + 1) * 64],
                )
                nc.sync.dma_start(
                    out_d[m, :, 128 + hc * 64 : 128 + (hc + 1) * 64],
                    out_sb[m][:, 128 + hc * 64 : 128 + (hc + 1) * 64],
                )

        # ---- the ne loop (x1 prefetched one iteration ahead) -----------
        def emit_x1(ne):
            x1 = []
            for m in range(2):
                o = ap.tile([128, R], F32R, tag=f"x1_{m}", name=f"x1_{m}")
                nc.scalar.activation(
                    o[:], uT[m][:], AF.Silu, bias=vb1[m][:, ne : ne + 1]
                )
                x1.append(o)
            return x1

        def emit_amr(ne, g3sb):
            Gm = gm_ctx["Gm"]
            for b2 in range(2):
                scr = sp.tile([128, PMAX], F32, tag="scr", name="scr")
                nc.vector.affine_mul_reduce(
                    out=scr[:],
                    accum_out=aggT[:, ne * 2 + b2 : ne * 2 + b2 + 1],
                    in0=g3sb[:, b2 * PMAX : (b2 + 1) * PMAX],
                    in1=Gm[:, b2 * PMAX : (b2 + 1) * PMAX],
                    scale=1.0,
                    bias=bia[:, PE_B3 : PE_B3 + 1],
                )

        pending = []
        x1 = x1_first
        for ne in range(NE):
            x1_next = emit_x1(ne + 1) if ne + 1 < NE else None
            x2 = []
            for m in range(2):
                ps = psB.tile([128, R], F32, tag="big", name="ps2")
                for rc in range(2):
                    for k in range(2):
                        nc.tensor.matmul(
                            ps[:, rc * 512 : (rc + 1) * 512],
                            peW2[k][m],
                            x1[k][:, rc * 512 : (rc + 1) * 512],
                            start=(k == 0),
                            stop=(k == 1),
                        )
                o = ap.tile([128, R], F32R, tag=f"x2_{m}", name=f"x2_{m}")
                nc.scalar.activation(
                    o[:], ps[:], AF.Silu, bias=bia[:, PE_B2 + m : PE_B2 + m + 1]
                )
                x2.append(o)
            ps3 = psS.tile([128, 512], F32, tag="small", name="ps3")
            scatter_mm(ps3, peW3, x2)
            g3sb = ap.tile([128, 512], F16, tag="g3sb", name="g3sb", bufs=20)
            nc.vector.tensor_copy(g3sb[:], ps3[:])
            if "Gm" in gm_ctx:
                for pne, pg in pending[:4]:
                    emit_amr(pne, pg)
                del pending[:4]
                emit_amr(ne, g3sb)
            else:
                pending.append((ne, g3sb))
            if ne >= 6 and (ne - 6) % 2 == 0 and (ne - 6) // 2 < len(gm_stages):
                gm_stages[(ne - 6) // 2]()
            x1 = x1_next
            if ne == 33:
                final_half(0)
        final_half(1)

    nc.compile()
    return nc


def _get_nc():
    if "nc" not in _CACHE:
        _CACHE["nc"] = _build()
    return _CACHE["nc"]


# --------------------------------------------------------------------------
# entry point
# --------------------------------------------------------------------------

def kernel(
    h,
    z,
    pos,
    mask,
    e_feat,
    z_emb,
    gm_W1,
    gm_b1,
    gm_W2,
    gm_b2,
    gm_W3,
    gm_b3,
    pe_W1,
    pe_b1,
    pe_W2,
    pe_b2,
    pe_W3,
    pe_b3,
    op_W1,
    op_b1,
    op_W2,
    op_b2,
    absorber_index=0,
):
    h = np.asarray(h, np.float32)
    z = np.asarray(z).astype(np.int64)
    pos = np.asarray(pos, np.float32)
    mask = np.asarray(mask).astype(bool)
    e_feat = np.asarray(e_feat, np.float32)
    z_emb = np.asarray(z_emb, np.float32)
    gm_W1 = np.asarray(gm_W1, np.float32)
    gm_b1 = np.asarray(gm_b1, np.float32)
    gm_W2 = np.asarray(gm_W2, np.float32)
    gm_b2 = np.asarray(gm_b2, np.float32)
    gm_W3 = np.asarray(gm_W3, np.float32)
    gm_b3 = np.asarray(gm_b3, np.float32)
    pe_W1 = np.asarray(pe_W1, np.float32)
    pe_b1 = np.asarray(pe_b1, np.float32)
    pe_W2 = np.asarray(pe_W2, np.float32)
    pe_b2 = np.asarray(pe_b2, np.float32)
    pe_W3 = np.asarray(pe_W3, np.float32)
    pe_b3 = np.asarray(pe_b3, np.float32)
    op_W1 = np.asarray(op_W1, np.float32)
    op_b1 = np.asarray(op_b1, np.float32)
    op_W2 = np.asarray(op_W2, np.float32)
    op_b2 = np.asarray(op_b2, np.float32)

    geom_in, ejk, pmask = _host_prep(h, z, pos, mask, z_emb, absorber_index)

    # v[ne] = e_feat @ pe_W1[64:] + pe_b1, the ne-dependent layer-1 bias
    vb1_full = (e_feat @ pe_W1[2 * ZEMB :] + pe_b1).astype(np.float32)  # [NE, PH]
    vb1 = vb1_full.T.reshape(2, 128, NE)

    biases = np.zeros((128, NBIAS), np.float32)
    biases[:, GM_B1] = gm_b1[:128]
    biases[:, GM_B1 + 1] = gm_b1[128:]
    biases[:, GM_B2] = gm_b2[:128]
    biases[:, GM_B2 + 1] = gm_b2[128:]
    biases[0:64, GM_B3] = gm_b3
    biases[64:128, GM_B3] = gm_b3
    biases[:, PE_B2] = pe_b2[:128]
    biases[:, PE_B2 + 1] = pe_b2[128:]
    biases[0:64, PE_B3] = pe_b3
    biases[64:128, PE_B3] = pe_b3
    biases[:, OP_B1] = op_b1[:128]
    biases[:, OP_B1 + 1] = op_b1[128:]
    biases[:, OP_B2] = op_b2[:128]
    biases[:, OP_B2 + 1] = op_b2[128:]

    wpack = np.zeros((128, WCOLS), np.float32)

    def put(c0, w):  # w: [K, M], tiles of [<=128 rows, 128 cols]
        kk, mm = w.shape
        for k in range(0, kk, 128):
            ksz = min(128, kk - k)
            for m in range(0, mm, 128):
                msz = min(128, mm - m)
                col = c0 + (k // 128) * mm + m
                wpack[0:ksz, col : col + msz] = w[k : k + ksz, m : m + msz]

    def put_rc(c0, w):  # w [256, 64] -> 4 zero-padded [128,128] tiles
        for k in range(2):
            for rc in range(2):
                col = c0 + k * 256 + rc * 128
                wpack[0:128, col + rc * 64 : col + rc * 64 + 64] = w[
                    k * 128 : (k + 1) * 128, :
                ]

    put(W_PEW2, pe_W2)
    put_rc(W_PEW3, pe_W3)
    put_rc(W_GMW3, gm_W3)

    put(W_GMW1, gm_W1)
    put(W_GMW2, gm_W2)

    put(W_OPW1, op_W1)  # rows 0:64, rows 64:128 stay zero
    wpack[64:128, W_OPW1H : W_OPW1H + 256] = wpack[0:64, W_OPW1 : W_OPW1 + 256]
    put(W_OPW2, op_W2)

    in_maps = []
    for c in range(N_CORES):
        sl = slice(c * BPC, (c + 1) * BPC)
        gi = geom_in[sl]  # [BPC, P, 353]
        ginT = np.ascontiguousarray(gi.reshape(R, GIN).T)  # [353, R]
        dpack = np.zeros((128, DCOLS), np.float32)
        dpack[0:64, D_EJK : D_EJK + R] = ejk[sl].reshape(R, 2 * ZEMB).T
        dpack[0:64, D_PW1 : D_PW1 + PH] = pe_W1[: 2 * ZEMB]
        dpack[:, D_VB1 : D_VB1 + NE] = vb1[0]
        dpack[:, D_VB1 + NE : D_VB1 + 2 * NE] = vb1[1]
        dpack[:, D_BIA : D_BIA + NBIAS] = biases
        # pmask in the [128=(rc,s), 512=(b2,p)] layout: row rc*64+s holds
        # sample (2*rc+b2)'s mask in column block b2
        pmc = pmask[sl].astype(np.float32)  # [4, 256]
        pm2 = np.zeros((128, 512), np.float32)
        for rc in range(2):
            for b2 in range(2):
                pm2[rc * 64 : (rc + 1) * 64, b2 * 256 : (b2 + 1) * 256] = pmc[
                    2 * rc + b2
                ][None, :]
        in_maps.append({"ginT": ginT, "dpack": dpack, "wpack": wpack, "pmask": pm2})

    nc = _get_nc()
    res = run_bass_kernel_spmd(nc, in_maps, list(range(N_CORES)), trace=TRACE)
    _CACHE["last_result"] = res

    out = np.empty((B, NE, OUT), np.float32)
    for c in range(N_CORES):
        oc = res.results[c]["out"]  # [2, 128, 256]; col = rc*128 + ne*2 + b2
        oc = oc.reshape(OUT, 2, NE, 2)  # [o, rc, ne, b2]
        out[c * BPC : (c + 1) * BPC] = oc.transpose(1, 3, 2, 0).reshape(BPC, NE, OUT)
    return out


# revision 20
# speedup vs baseline: 1.1392x; 1.0412x over previous
"""Trainium2 Bass kernel for nn_AbsorberPathAggregator.

Contract: kernel(**inputs) takes the FULL unsharded inputs (as produced by
setup_inputs()) and returns the FULL [B, NE, OUT] float32 output.

Strategy (pure data parallel over B across 8 NeuronCores, 4 samples/core):
  - Host (numpy): pair enumeration (argsort of valid triu pairs), gathers,
    RBF features -> geom_inT [353, 1024] per core; ejkT [64, 1024]; pmask.
    This is cheap index/bookkeeping work; all FLOP-heavy MLPs run on device.
  - Device per core, activations transposed [feature, row], rows = (b, p):
      G = gm-MLP(geom_inT) masked by pmask        (geometry path)
      u = pe_W1[:64].T @ ejkT                     (pair part of elem layer 1)
      for ne in 0..63:  x1 = Silu(u + v[ne]); x2 = Silu(W2.T x1)
                        g3 = W3.T x2;  agg[:, ne] += sum_p (g3+b3)*G
      out = Silu(opW1.T agg + b1) -> opW2 + b2 -> DMA out
    g3/G/agg live in a [128=(rc,s), 512=(b2,p)] layout (rc = sample-pair,
    placed via PE column tiling) so the fused affine_mul_reduce epilogue is
    2 ops of [128, 256] per ne. Matmuls use float32r (full-rate fp32-reduced);
    Silu runs on ScalarE with the free per-partition bias operand (ScalarE is
    the bottleneck engine at ~273 us busy).
"""

import sys

for _p in (
    "/root/.axon_site",
    "/root/.axon_site/_ro/trn_rl_repo",
    "/root/.axon_site/_ro/pypackages",
    "/opt/trn_rl_repo",
):
    if _p not in sys.path:
        sys.path.append(_p)

import numpy as np

from contextlib import ExitStack

import concourse.tile as tile
from concourse import bacc, mybir
from concourse.bass_utils import run_bass_kernel_spmd

F32 = mybir.dt.float32
F32R = mybir.dt.float32r
F16 = mybir.dt.float16
AF = mybir.ActivationFunctionType
ALU = mybir.AluOpType

B, N, H = 32, 64, 128
RBF_DIM, GH, SCATTER, OUT = 32, 256, 64, 256
CUTOFF, PMAX = 6.0, 256
ZEMB, EDIM, NE, PH = 32, 32, 64, 256
GIN = 2 * H + 3 * RBF_DIM + 1  # 353

N_CORES = 8
BPC = B // N_CORES  # 4 samples per core
R = BPC * PMAX  # 1024 rows per core

TRACE = False  # set by test harness for NTFF profiling
_CACHE = {}


# --------------------------------------------------------------------------
# host-side preprocessing (numpy, mirrors the reference's indexing exactly)
# --------------------------------------------------------------------------

def _rbf(x):
    centers = np.linspace(0.0, CUTOFF, RBF_DIM, dtype=np.float32)
    delta = CUTOFF / (RBF_DIM - 1)
    gamma = 1.0 / (delta * delta + 1e-12)
    d = x[..., None] - centers
    return np.exp((-gamma) * d * d).astype(np.float32)


def _norm(v):
    return np.sqrt((v * v).sum(-1) + np.float32(1e-12))


def _host_prep(h, z, pos, mask, z_emb, absorber_index):
    ai = int(absorber_index)
    pos0 = pos[:, ai][:, None, :]  # [B,1,3]
    r = _norm(pos - pos0)  # [B,N]
    valid = mask & (np.arange(N)[None, :] != ai) & (r <= np.float32(CUTOFF))
    ju, ku = np.triu_indices(N, k=1)
    pv = valid[:, ju] & valid[:, ku]  # [B, Np]
    order = np.argsort(~pv, axis=1, kind="stable")[:, :PMAX]
    pmask = np.take_along_axis(pv, order, axis=1)  # [B,PMAX]
    j_idx = np.where(pmask, ju[order], 0)
    k_idx = np.where(pmask, ku[order], 0)
    bidx = np.arange(B)[:, None]
    hj, hk = h[bidx, j_idx], h[bidx, k_idx]  # [B,P,H]
    posj, posk = pos[bidx, j_idx], pos[bidx, k_idx]  # [B,P,3]
    vj, vk, vjk = posj - pos0, posk - pos0, posk - posj
    r0j, r0k, rjk = _norm(vj), _norm(vk), _norm(vjk)
    uj = vj / np.maximum(r0j[..., None], np.float32(1e-8))
    uk = vk / np.maximum(r0k[..., None], np.float32(1e-8))
    cosang = np.clip((uj * uk).sum(-1, keepdims=True), -1.0, 1.0).astype(np.float32)
    geom_in = np.concatenate(
        [
            hj,
            hk,
            _rbf(np.minimum(r0j, np.float32(CUTOFF))),
            _rbf(np.minimum(r0k, np.float32(CUTOFF))),
            _rbf(np.minimum(rjk, np.float32(CUTOFF))),
            cosang,
        ],
        axis=-1,
    ).astype(np.float32)  # [B, P, 353]
    ejk = np.concatenate([z_emb[z[bidx, j_idx]], z_emb[z[bidx, k_idx]]], axis=-1)
    return geom_in, ejk.astype(np.float32), pmask


# --------------------------------------------------------------------------
# device kernel
# --------------------------------------------------------------------------

# bias-pack column indices (within the dpack bias block)
GM_B1, GM_B2, GM_B3, PE_B2, PE_B3, OP_B1, OP_B2 = 0, 2, 4, 5, 7, 8, 10
NBIAS = 12

# epack [64, 1280] fp16: cols 0:256 = pe_W1[:64], cols 256:1280 = ejkT.
# dpack [128, DCOLS] f32: vb1 (2 mh halves of 64) then biases.
E_PW1 = 0
E_EJK = 256
ECOLS = 256 + R
D_VB1 = 0
D_BIA = 128
DCOLS = 128 + NBIAS

# wpack [128, WCOLS] column layout: peW2(512) peW3rc(512) gmW1(768)
# gmW2(512) gmW3rc(512) opW1(256, duplicated on rows 64:128) opW2(512).
# W3 tiles are stored zero-padded to [128,128] as [W3|0] (rc0) and [0|W3]
# (rc1) so the scatter layer lands both rc halves in one [128,512] psum
# without PE tile_position.
W_PEW2 = 0
W_PEW3 = 512
W_GMW1 = 1024
W_GMW2 = 1792
W_GMW3 = 2304
W_OPW1 = 2816
W_OPW1H = 3072
W_OPW2 = 3328
WCOLS = 3840
W_SPLIT = 1024  # pe weights first; loaded ahead of the rest


def _build():
    nc = bacc.Bacc("TRN2", target_bir_lowering=False, debug=False, num_devices=N_CORES)

    ginT_d = nc.dram_tensor("ginT", [GIN, R], F32R, kind="ExternalInput").ap()
    pmask_d = nc.dram_tensor("pmask", [128, 512], F32, kind="ExternalInput").ap()
    epack_d = nc.dram_tensor("epack", [64, ECOLS], F16, kind="ExternalInput").ap()
    dpack_d = nc.dram_tensor("dpack", [128, DCOLS], F32, kind="ExternalInput").ap()
    wpack_d = nc.dram_tensor("wpack", [128, WCOLS], F32R, kind="ExternalInput").ap()
    out_d = nc.dram_tensor("out", [2, 128, 256], F32, kind="ExternalOutput").ap()

    K1 = [(0, 128), (128, 128), (256, GIN - 256)]  # gm layer-1 k tiles

    with tile.TileContext(nc) as tc, ExitStack() as ctx:
        wp = ctx.enter_context(tc.tile_pool(name="wp", bufs=1))
        ap = ctx.enter_context(tc.tile_pool(name="ap", bufs=3))
        sp = ctx.enter_context(tc.tile_pool(name="sp", bufs=2))
        psB = ctx.enter_context(tc.tile_pool(name="psB", bufs=3, space="PSUM"))
        psS = ctx.enter_context(tc.tile_pool(name="psS", bufs=2, space="PSUM"))

        # ---- loads (sync HWDGE; dpack is on the critical path) ----------
        epack = wp.tile([64, ECOLS], F16, tag="epack")
        nc.sync.dma_start(epack[:], epack_d)
        dpack = wp.tile([128, DCOLS], F32, tag="dpack")
        nc.sync.dma_start(dpack[:], dpack_d)
        wpack = wp.tile([128, WCOLS], F32R, tag="wpack")
        nc.sync.dma_start(wpack[:, 0:W_SPLIT], wpack_d[:, 0:W_SPLIT])
        nc.sync.dma_start(wpack[:, W_SPLIT:W_GMW2], wpack_d[:, W_SPLIT:W_GMW2])
        nc.sync.dma_start(wpack[:, W_GMW2:], wpack_d[:, W_GMW2:])
        pmask2t = wp.tile([128, 512], F32, tag="pmask2t")
        nc.sync.dma_start(pmask2t[:], pmask_d)
        pmask2 = pmask2t[:]

        ejkT = epack[:, E_EJK : E_EJK + R]
        pw1e = [epack[:, E_PW1 + m * 128 : E_PW1 + (m + 1) * 128] for m in range(2)]
        vb1 = [dpack[:, D_VB1 + m * NE : D_VB1 + (m + 1) * NE] for m in range(2)]
        bia = dpack[:, D_BIA : D_BIA + NBIAS]

        def wtile(c0, ksz, m):
            return wpack[0:ksz, c0 + m * 128 : c0 + (m + 1) * 128]

        peW2 = [[wtile(W_PEW2 + k * 256, 128, m) for m in range(2)] for k in range(2)]
        peW3 = [
            [
                wpack[0:128, W_PEW3 + k * 256 + rc * 128 : W_PEW3 + k * 256 + (rc + 1) * 128]
                for rc in range(2)
            ]
            for k in range(2)
        ]
        gmW1 = [
            [wtile(W_GMW1 + k * 256, K1[k][1], m) for m in range(2)] for k in range(3)
        ]
        gmW2 = [[wtile(W_GMW2 + k * 256, 128, m) for m in range(2)] for k in range(2)]
        gmW3 = [
            [
                wpack[0:128, W_GMW3 + k * 256 + rc * 128 : W_GMW3 + k * 256 + (rc + 1) * 128]
                for rc in range(2)
            ]
            for k in range(2)
        ]
        opW1 = [
            wpack[0:128, W_OPW1 + m * 128 : W_OPW1 + (m + 1) * 128] for m in range(2)
        ]
        opW1h = [
            wpack[0:128, W_OPW1H + m * 128 : W_OPW1H + (m + 1) * 128] for m in range(2)
        ]
        opW2 = [[wtile(W_OPW2 + k * 256, 128, m) for m in range(2)] for k in range(2)]

        ginT = []
        for k, (k0, ksz) in enumerate(K1):
            t = wp.tile([ksz, R], F32R, tag=f"ginT{k}", name=f"ginT{k}")
            nc.scalar.dma_start(t[:], ginT_d[k0 : k0 + ksz, :])
            ginT.append(t)

        # ---- u = pw1e.T @ ejkT  (pair part of elem layer 1) ------------
        # uT2 holds both mh halves side by side: cols m*R + (b,p)
        uT2 = wp.tile([128, 2 * R], F32, tag="uT2")
        for m in range(2):
            for rc in range(2):
                psu = psS.tile([128, 512], F32, tag="small", name="psu")
                nc.tensor.matmul(
                    psu[:],
                    pw1e[m],
                    ejkT[:, rc * 512 : (rc + 1) * 512],
                    start=True,
                    stop=True,
                )
                nc.vector.tensor_copy(
                    uT2[:, m * R + rc * 512 : m * R + (rc + 1) * 512], psu[:]
                )

        def emit_x1(ne):
            # ne-bias pre-add on VectorE, then ONE [128, 2R] Silu on ScalarE
            x1p = ap.tile([128, 2 * R], F32R, tag="x1p", name="x1p")
            for m in range(2):
                nc.vector.tensor_scalar_add(
                    x1p[:, m * R : (m + 1) * R],
                    uT2[:, m * R : (m + 1) * R],
                    vb1[m][:, ne : ne + 1],
                )
            o = ap.tile([128, 2 * R], F32R, tag="x1", name="x1")
            nc.scalar.activation(o[:], x1p[:], AF.Silu)
            return [o[:, 0:R], o[:, R : 2 * R]]

        x1_first = emit_x1(0)

        # scatter-layer matmul: write [64 x 512] results of both rc halves
        # into one [128=(rc,s), 512=(b2,p)] psum tile via PE column tiling
        def scatter_mm(ps, weights, rhs_tiles):
            for rc in range(2):
                for k in range(2):
                    nc.tensor.matmul(
                        ps[:],
                        weights[k][rc],
                        rhs_tiles[k][:, rc * 512 : (rc + 1) * 512],
                        start=(rc == 0 and k == 0),
                        stop=(rc == 1 and k == 1),
                    )

        # ---- geometry MLP (emission staged into the ne loop below) -----
        gm_ctx = {}

        def mlp_half(rhs_tiles, weights, bias_col, out_name):
            ps = psB.tile([128, R], F32, tag="big", name="ps_mlp")
            nk = len(rhs_tiles)
            for rc in range(2):
                for k in range(nk):
                    nc.tensor.matmul(
                        ps[:, rc * 512 : (rc + 1) * 512],
                        weights[k],
                        rhs_tiles[k][:, rc * 512 : (rc + 1) * 512],
                        start=(k == 0),
                        stop=(k == nk - 1),
                    )
            o = wp.tile([128, R], F32R, tag=out_name, name=out_name)
            nc.scalar.activation(
                o[:], ps[:], AF.Silu, bias=bia[:, bias_col : bias_col + 1]
            )
            return o

        def gm_stage_0():
            gm_ctx["x1g0"] = mlp_half(ginT, [w[0] for w in gmW1], GM_B1, "gx1_0")

        def gm_stage_1():
            gm_ctx["x1g1"] = mlp_half(ginT, [w[1] for w in gmW1], GM_B1 + 1, "gx1_1")

        def gm_stage_2():
            x1g = [gm_ctx["x1g0"], gm_ctx["x1g1"]]
            gm_ctx["x2g0"] = mlp_half(x1g, [w[0] for w in gmW2], GM_B2, "gx2_0")

        def gm_stage_3():
            x1g = [gm_ctx["x1g0"], gm_ctx["x1g1"]]
            gm_ctx["x2g1"] = mlp_half(x1g, [w[1] for w in gmW2], GM_B2 + 1, "gx2_1")

        def gm_stage_4():
            x2g = [gm_ctx["x2g0"], gm_ctx["x2g1"]]
            psG = psS.tile([128, 512], F32, tag="small", name="psG")
            scatter_mm(psG, gmW3, x2g)
            Gtmp = sp.tile([128, 512], F32, tag="Gtmp", name="Gtmp")
            nc.vector.tensor_scalar_add(Gtmp[:], psG[:], bia[:, GM_B3 : GM_B3 + 1])
            Gm = wp.tile([128, 512], F32, tag="Gm", name="Gm")
            nc.vector.tensor_mul(Gm[:], Gtmp[:], pmask2)
            gm_ctx["Gm"] = Gm

        gm_stages = [gm_stage_0, gm_stage_1, gm_stage_2, gm_stage_3, gm_stage_4]

        # ---- final output MLP, one 64-column (of 128) chunk at a time --
        # aggT [128=(rc,s), 128=(ne*2+b2)]
        aggT = wp.tile([128, NE * 2], F32, tag="aggT")
        out_sb = [sp.tile([128, 256], F32, tag=f"oT_{m}", name=f"oT_{m}") for m in range(2)]

        def final_half(hc):
            cs = slice(hc * 64, (hc + 1) * 64)
            aggR = sp.tile([128, 64], F32R, tag="aggR", name="aggR", bufs=2)
            nc.vector.tensor_copy(aggR[:], aggT[:, cs])
            f1 = []
            for m in range(2):
                ps = psS.tile([128, 128], F32, tag="small", name="psf1")
                nc.tensor.matmul(ps[:, 0:64], opW1[m], aggR[:], start=True, stop=True)
                nc.tensor.matmul(
                    ps[:, 64:128], opW1h[m], aggR[:], start=True, stop=True
                )
                o = sp.tile([128, 128], F32R, tag=f"f1_{m}", name=f"f1_{m}")
                nc.scalar.activation(
                    o[:], ps[:], AF.Silu, bias=bia[:, OP_B1 + m : OP_B1 + m + 1]
                )
                f1.append(o)
            for m in range(2):
                ps = psS.tile([128, 128], F32, tag="small", name="psf2")
                for k in range(2):
                    nc.tensor.matmul(
                        ps[:], opW2[k][m], f1[k][:], start=(k == 0), stop=(k == 1)
                    )
                for rc in range(2):
                    nc.vector.tensor_scalar_add(
                        out_sb[m][:, rc * 128 + hc * 64 : rc * 128 + (hc + 1) * 64],
                        ps[:, rc * 64 : (rc + 1) * 64],
                        bia[:, OP_B2 + m : OP_B2 + m + 1],
                    )
                nc.sync.dma_start(
                    out_d[m, :, hc * 64 : (hc + 1) * 64],
                    out_sb[m][:, hc * 64 : (hc + 1) * 64],
                )
                nc.sync.dma_start(
                    out_d[m, :, 128 + hc * 64 : 128 + (hc + 1) * 64],
                    out_sb[m][:, 128 + hc * 64 : 128 + (hc + 1) * 64],
                )

        # ---- the ne loop (x1 prefetched one iteration ahead) -----------
        def emit_amr(ne, g3sb):
            Gm = gm_ctx["Gm"]
            for b2 in range(2):
                scr = sp.tile([128, PMAX], F32, tag="scr", name="scr")
                nc.vector.affine_mul_reduce(
                    out=scr[:],
                    accum_out=aggT[:, ne * 2 + b2 : ne * 2 + b2 + 1],
                    in0=g3sb[:, b2 * PMAX : (b2 + 1) * PMAX],
                    in1=Gm[:, b2 * PMAX : (b2 + 1) * PMAX],
                    scale=1.0,
                    bias=bia[:, PE_B3 : PE_B3 + 1],
                )

        pending = []
        x1 = x1_first
        for ne in range(NE):
            x1_next = emit_x1(ne + 1) if ne + 1 < NE else None
            x2 = []
            for m in range(2):
                ps = psB.tile([128, R], F32, tag="big", name="ps2")
                for rc in range(2):
                    for k in range(2):
                        nc.tensor.matmul(
                            ps[:, rc * 512 : (rc + 1) * 512],
                            peW2[k][m],
                            x1[k][:, rc * 512 : (rc + 1) * 512],
                            start=(k == 0),
                            stop=(k == 1),
                        )
                o = ap.tile([128, R], F32R, tag=f"x2_{m}", name=f"x2_{m}")
                nc.scalar.activation(
                    o[:], ps[:], AF.Silu, bias=bia[:, PE_B2 + m : PE_B2 + m + 1]
                )
                x2.append(o)
            ps3 = psS.tile([128, 512], F32, tag="small", name="ps3")
            scatter_mm(ps3, peW3, x2)
            g3sb = ap.tile([128, 512], F16, tag="g3sb", name="g3sb", bufs=20)
            nc.vector.tensor_copy(g3sb[:], ps3[:])
            if "Gm" in gm_ctx:
                for pne, pg in pending[:2]:
                    emit_amr(pne, pg)
                del pending[:2]
                emit_amr(ne, g3sb)
            else:
                pending.append((ne, g3sb))
            if ne >= 6 and (ne - 6) % 2 == 0 and (ne - 6) // 2 < len(gm_stages):
                gm_stages[(ne - 6) // 2]()
            x1 = x1_next
            if ne == 33:
                final_half(0)
        final_half(1)

    nc.compile()
    return nc


def _get_nc():
    if "nc" not in _CACHE:
        _CACHE["nc"] = _build()
    return _CACHE["nc"]


# --------------------------------------------------------------------------
# entry point
# --------------------------------------------------------------------------

def kernel(
    h,
    z,
    pos,
    mask,
    e_feat,
    z_emb,
    gm_W1,
    gm_b1,
    gm_W2,
    gm_b2,
    gm_W3,
    gm_b3,
    pe_W1,
    pe_b1,
    pe_W2,
    pe_b2,
    pe_W3,
    pe_b3,
    op_W1,
    op_b1,
    op_W2,
    op_b2,
    absorber_index=0,
):
    h = np.asarray(h, np.float32)
    z = np.asarray(z).astype(np.int64)
    pos = np.asarray(pos, np.float32)
    mask = np.asarray(mask).astype(bool)
    e_feat = np.asarray(e_feat, np.float32)
    z_emb = np.asarray(z_emb, np.float32)
    gm_W1 = np.asarray(gm_W1, np.float32)
    gm_b1 = np.asarray(gm_b1, np.float32)
    gm_W2 = np.asarray(gm_W2, np.float32)
    gm_b2 = np.asarray(gm_b2, np.float32)
    gm_W3 = np.asarray(gm_W3, np.float32)
    gm_b3 = np.asarray(gm_b3, np.float32)
    pe_W1 = np.asarray(pe_W1, np.float32)
    pe_b1 = np.asarray(pe_b1, np.float32)
    pe_W2 = np.asarray(pe_W2, np.float32)
    pe_b2 = np.asarray(pe_b2, np.float32)
    pe_W3 = np.asarray(pe_W3, np.float32)
    pe_b3 = np.asarray(pe_b3, np.float32)
    op_W1 = np.asarray(op_W1, np.float32)
    op_b1 = np.asarray(op_b1, np.float32)
    op_W2 = np.asarray(op_W2, np.float32)
    op_b2 = np.asarray(op_b2, np.float32)

    geom_in, ejk, pmask = _host_prep(h, z, pos, mask, z_emb, absorber_index)

    # v[ne] = e_feat @ pe_W1[64:] + pe_b1, the ne-dependent layer-1 bias
    vb1_full = (e_feat @ pe_W1[2 * ZEMB :] + pe_b1).astype(np.float32)  # [NE, PH]
    vb1 = vb1_full.T.reshape(2, 128, NE)

    biases = np.zeros((128, NBIAS), np.float32)
    biases[:, GM_B1] = gm_b1[:128]
    biases[:, GM_B1 + 1] = gm_b1[128:]
    biases[:, GM_B2] = gm_b2[:128]
    biases[:, GM_B2 + 1] = gm_b2[128:]
    biases[0:64, GM_B3] = gm_b3
    biases[64:128, GM_B3] = gm_b3
    biases[:, PE_B2] = pe_b2[:128]
    biases[:, PE_B2 + 1] = pe_b2[128:]
    biases[0:64, PE_B3] = pe_b3
    biases[64:128, PE_B3] = pe_b3
    biases[:, OP_B1] = op_b1[:128]
    biases[:, OP_B1 + 1] = op_b1[128:]
    biases[:, OP_B2] = op_b2[:128]
    biases[:, OP_B2 + 1] = op_b2[128:]

    wpack = np.zeros((128, WCOLS), np.float32)

    def put(c0, w):  # w: [K, M], tiles of [<=128 rows, 128 cols]
        kk, mm = w.shape
        for k in range(0, kk, 128):
            ksz = min(128, kk - k)
            for m in range(0, mm, 128):
                msz = min(128, mm - m)
                col = c0 + (k // 128) * mm + m
                wpack[0:ksz, col : col + msz] = w[k : k + ksz, m : m + msz]

    def put_rc(c0, w):  # w [256, 64] -> 4 zero-padded [128,128] tiles
        for k in range(2):
            for rc in range(2):
                col = c0 + k * 256 + rc * 128
                wpack[0:128, col + rc * 64 : col + rc * 64 + 64] = w[
                    k * 128 : (k + 1) * 128, :
                ]

    put(W_PEW2, pe_W2)
    put_rc(W_PEW3, pe_W3)
    put_rc(W_GMW3, gm_W3)

    put(W_GMW1, gm_W1)
    put(W_GMW2, gm_W2)

    put(W_OPW1, op_W1)  # rows 0:64, rows 64:128 stay zero
    wpack[64:128, W_OPW1H : W_OPW1H + 256] = wpack[0:64, W_OPW1 : W_OPW1 + 256]
    put(W_OPW2, op_W2)

    in_maps = []
    for c in range(N_CORES):
        sl = slice(c * BPC, (c + 1) * BPC)
        gi = geom_in[sl]  # [BPC, P, 353]
        ginT = np.ascontiguousarray(gi.reshape(R, GIN).T)  # [353, R]
        epack = np.zeros((64, ECOLS), np.float16)
        epack[:, E_PW1 : E_PW1 + PH] = pe_W1[: 2 * ZEMB]
        epack[:, E_EJK : E_EJK + R] = ejk[sl].reshape(R, 2 * ZEMB).T
        dpack = np.zeros((128, DCOLS), np.float32)
        dpack[:, D_VB1 : D_VB1 + NE] = vb1[0]
        dpack[:, D_VB1 + NE : D_VB1 + 2 * NE] = vb1[1]
        dpack[:, D_BIA : D_BIA + NBIAS] = biases
        # pmask in the [128=(rc,s), 512=(b2,p)] layout: row rc*64+s holds
        # sample (2*rc+b2)'s mask in column block b2
        pmc = pmask[sl].astype(np.float32)  # [4, 256]
        pm2 = np.zeros((128, 512), np.float32)
        for rc in range(2):
            for b2 in range(2):
                pm2[rc * 64 : (rc + 1) * 64, b2 * 256 : (b2 + 1) * 256] = pmc[
                    2 * rc + b2
                ][None, :]
        in_maps.append({"ginT": ginT, "epack": epack, "dpack": dpack, "wpack": wpack, "pmask": pm2})

    nc = _get_nc()
    res = run_bass_kernel_spmd(nc, in_maps, list(range(N_CORES)), trace=TRACE)
    _CACHE["last_result"] = res

    out = np.empty((B, NE, OUT), np.float32)
    for c in range(N_CORES):
        oc = res.results[c]["out"]  # [2, 128, 256]; col = rc*128 + ne*2 + b2
        oc = oc.reshape(OUT, 2, NE, 2)  # [o, rc, ne, b2]
        out[c * BPC : (c + 1) * BPC] = oc.transpose(1, 3, 2, 0).reshape(BPC, NE, OUT)
    return out
